# revision 4
# baseline (speedup 1.0000x reference)
"""BalanceCrossEntropyLoss on 8 trn2 NeuronCores.

Full (unsharded) inputs in, full output (scalar) out. Data-parallel over N:
each core takes 2 of the 16 images. The global top-k negative-loss sum is
computed threshold-style: a per-partition bisection on an all-gathered sample
estimates the k-th-largest threshold tau, then one exact masked sum/count pass
plus the correction  sum_topk = S(tau) + (k - C(tau)) * tau  (error is
quadratic in the tau estimation error; ~1e-5 relative here).

Cross-core exchanges use remote_dma_broadcast (SBUF-to-SBUF peer DMA with
semaphore signaling) instead of collective_compute: the ncfw collective path
costs ~75us of one-time setup per execution, peer DMA is ~2us. Sender s
writes receiver r's slot s^r via XOR-addressed singleton broadcasts, which
keeps the SPMD program uniform with compile-time APs.
"""
import sys, types

sys.path.insert(0, "/opt/trn_rl_repo")
import numpy as np

import concourse.bass as bass
import concourse.bacc as bacc
import concourse.mybir as mybir
import concourse.tile as tile
from concourse import library_config
from concourse.bass_utils import run_bass_kernel_spmd

F32 = mybir.dt.float32
OP = mybir.AluOpType
AF = mybir.ActivationFunctionType

N_CORES = 8
N, H, W = 16, 640, 640
P = 128                      # SBUF partitions
FREE = (N // N_CORES) * H * W // P   # 6400 columns per core
CHUNK = 1600                 # streaming chunk (4 chunks)
N_CH = FREE // CHUNK
SAMPLE_STRIDE = 64
N_SAMP = FREE // SAMPLE_STRIDE       # 100 sample columns per core
PAY = N_SAMP + 4             # payload cols: samples + pos_cnt, pos_sum', mask_sum, pad
N_TOTAL = float(N * H * W)   # 6553600 elements globally
NEG_RATIO = 3.0
EPS = 1e-6
# loss values -ln(1-p) lie in (0.01, 4.606] for p in [0.01, 0.99]; we search on
# negated values R' in [-4.75, 0]
LO = -4.75
N_ITER = 11

TRACE = False
_NC_CACHE = {}


def _ensure_trace_hook():
    import antenv
    if "antenv.axon_hooks" not in sys.modules:
        _hooks = types.ModuleType("antenv.axon_hooks")
        _hooks._hook = None
        def _set(h): _hooks._hook = h
        def _get(): return _hooks._hook
        _hooks.set_axon_ntff_profile_hook = _set
        _hooks.get_axon_ntff_profile_hook = _get
        sys.modules["antenv.axon_hooks"] = _hooks
        antenv.axon_hooks = _hooks
        from trn_agent_boot.trn_boot import _ntff_profile_via_ctypes
        _set(_ntff_profile_via_ctypes("/opt/axon/libaxon_pjrt.so"))


def _exchange(nc, tc, src, dst, n_slots, slot_cols, rsem, lsem, psem, expect,
              marker_eng):
    """All-gather src [P, slot_cols] into dst [P, n_slots*slot_cols] across
    cores via XOR-addressed singleton remote_dma_broadcasts. Receiver slot j
    holds the payload of core (self_id ^ j). Explicit prep-sem ordering: the
    trigger must not fire before all desc-gens committed (Tile reorders
    nosync-dep-linked instructions inside critical sections)."""
    with tc.tile_critical():
        for j in range(N_CORES):
            rdests = [None] * N_CORES
            rdests[j] = (0, j)
            nc.gpsimd.remote_dma_broadcast(
                dst[:, j * slot_cols:(j + 1) * slot_cols], src[:],
                remote_sem=rsem, local_sem=lsem, rdests=rdests
            ).then_inc(psem, 1)
        nc.gpsimd.wait_ge(psem, N_CORES)
        nc.gpsimd.trigger_dma(count=N_CORES)
        marker_eng.wait_ge(rsem, expect)
        marker_eng.tensor_copy(dst[0:1, 0:1], dst[0:1, 0:1])


def build():
    nc = bacc.Bacc("TRN2", target_bir_lowering=False, debug=False,
                   num_devices=N_CORES)
    pred = nc.dram_tensor("pred", [P, FREE], F32, kind="ExternalInput").ap()
    gt = nc.dram_tensor("gt", [P, FREE], F32, kind="ExternalInput").ap()
    mask = nc.dram_tensor("mask", [P, FREE], F32, kind="ExternalInput").ap()
    out = nc.dram_tensor("out", [1, 8], F32, kind="ExternalOutput").ap()

    rsem1 = nc.alloc_semaphore("rsem1")
    rsem2 = nc.alloc_semaphore("rsem2")
    lsem = nc.alloc_semaphore("lsem")
    psem1 = nc.alloc_semaphore("psem1")
    psem2 = nc.alloc_semaphore("psem2")

    with tile.TileContext(nc) as tc:
        with tc.tile_pool(name="io", bufs=2) as io, \
             tc.tile_pool(name="mids", bufs=2) as mids, \
             tc.tile_pool(name="res", bufs=1) as res, \
             tc.tile_pool(name="small", bufs=1) as small, \
             tc.tile_pool(name="psum", bufs=2, space="PSUM") as psum:

            # gpsimd ucode with remote_dma desc-gen + tensor_tensor
            nc.gpsimd.load_library(library_config.proxy)

            # ---- persistent tiles ----
            Rp = res.tile([P, FREE], F32)        # resident R' = neg * ln(1-p) <= 0
            junk6 = res.tile([P, FREE], F32)     # big scratch
            ones = small.tile([P, P], F32)
            nc.vector.memset(ones[:], 1.0)
            pcnt_c = small.tile([P, N_CH], F32)  # per-chunk accums
            psumc = small.tile([P, N_CH], F32)
            mcnt_c = small.tile([P, N_CH], F32)

            # ---- streaming phase ----
            for ch in range(N_CH):
                sl = slice(ch * CHUNK, (ch + 1) * CHUNK)
                pt = io.tile([P, CHUNK], F32, tag="pred")
                gtt = io.tile([P, CHUNK], F32, tag="gt")
                mt = io.tile([P, CHUNK], F32, tag="mask")
                nc.sync.dma_start(pt[:], pred[:, sl])
                nc.sync.dma_start(gtt[:], gt[:, sl])
                nc.sync.dma_start(mt[:], mask[:, sl])
                lp = mids.tile([P, CHUNK], F32, tag="lp")
                lq = mids.tile([P, CHUNK], F32, tag="lq")
                # ACT: ln(p), ln(1-p), and sum(mask) via Copy-accum
                nc.scalar.activation(lp[:], pt[:], AF.Ln, bias=0.0, scale=1.0)
                nc.scalar.activation(lq[:], pt[:], AF.Ln, bias=1.0, scale=-1.0)
                junka = mids.tile([P, CHUNK], F32, tag="junka")
                nc.scalar.activation(junka[:], mt[:], AF.Copy, bias=0.0,
                                     scale=1.0, accum_out=mcnt_c[:, ch:ch + 1])
                # DVE: pm = gt*mask (accum -> pos_cnt)
                pm = mids.tile([P, CHUNK], F32, tag="pm")
                nc.vector.scalar_tensor_tensor(
                    pm[:], gtt[:], 0.0, mt[:], OP.bypass, OP.mult,
                    accum_out=pcnt_c[:, ch:ch + 1])
                # GPSIMD: nm = mask - pm
                nm = mids.tile([P, CHUNK], F32, tag="nm")
                nc.gpsimd.tensor_tensor(nm[:], mt[:], pm[:], OP.subtract)
                # DVE: pos-loss partial: (lp)*pm, accum -> pos_sum' (= -pos_sum)
                junkb = mids.tile([P, CHUNK], F32, tag="junkb")
                nc.vector.scalar_tensor_tensor(
                    junkb[:], lp[:], 0.0, pm[:], OP.bypass, OP.mult,
                    accum_out=psumc[:, ch:ch + 1])
                # DVE: R' = lq * nm  (resident)
                nc.vector.scalar_tensor_tensor(
                    Rp[:, sl], lq[:], 0.0, nm[:], OP.bypass, OP.mult)

            # ---- reduce per-chunk accums, pack exchange-1 payload ----
            pay = small.tile([P, PAY], F32)
            # sample: every 64th column of R'
            samp_view = Rp[:].rearrange("p (n s) -> p n s", s=SAMPLE_STRIDE)[:, :, 0]
            nc.vector.tensor_copy(pay[:, 0:N_SAMP], samp_view)
            nc.vector.tensor_reduce(pay[:, N_SAMP:N_SAMP + 1], pcnt_c[:],
                                    axis=mybir.AxisListType.X, op=OP.add)
            nc.vector.tensor_reduce(pay[:, N_SAMP + 1:N_SAMP + 2], psumc[:],
                                    axis=mybir.AxisListType.X, op=OP.add)
            nc.vector.tensor_reduce(pay[:, N_SAMP + 2:N_SAMP + 3], mcnt_c[:],
                                    axis=mybir.AxisListType.X, op=OP.add)
            nc.vector.memset(pay[:, N_SAMP + 3:N_SAMP + 4], 0.0)

            gat = small.tile([P, N_CORES * PAY], F32)
            _exchange(nc, tc, pay, gat, N_CORES, PAY, rsem1, lsem, psem1,
                      expect=16, marker_eng=nc.vector)

            # views into the gathered payload
            gv = gat[:].rearrange("p (c j) -> p c j", c=N_CORES)
            G = gv[:, :, 0:N_SAMP]                 # [P, 8, 100] samples
            junkG = junk6[:, 0:N_CORES * PAY].rearrange(
                "p (c j) -> p c j", c=N_CORES)[:, :, 0:N_SAMP]

            # ---- global exact counts: pos_cnt, pos_sum', mask_sum ----
            st3 = small.tile([P, 3], F32)
            for i in range(3):
                nc.vector.tensor_reduce(st3[:, i:i + 1], gv[:, :, N_SAMP + i],
                                        axis=mybir.AxisListType.X, op=OP.add)
            ps3 = psum.tile([P, 3], F32)
            nc.tensor.matmul(ps3[:], ones[:], st3[:], start=True, stop=True)
            glob = small.tile([P, 8], F32)  # 0:pos_cnt 1:pos_sum' 2:mask_sum 3:neg_cnt 4:k 5:- 6:c0 7:scratch
            nc.vector.tensor_copy(glob[:, 0:3], ps3[:])
            # neg_cnt = mask_sum - pos_cnt
            nc.vector.tensor_tensor(glob[:, 3:4], glob[:, 2:3], glob[:, 0:1],
                                    OP.subtract)
            # k = min(neg_cnt, 3*pos_cnt)
            nc.vector.tensor_scalar(glob[:, 7:8], glob[:, 0:1], NEG_RATIO, None,
                                    OP.mult)
            nc.vector.tensor_tensor(glob[:, 4:5], glob[:, 3:4], glob[:, 7:8],
                                    OP.min)
            # c0_p = per-partition count of valid (nonzero) sampled negatives
            nc.vector.tensor_scalar(junkG, G, -1e-3, 0.0, OP.is_lt, OP.add,
                                    accum_out=glob[:, 6:7])
            # t_p = k * c0_p / neg_cnt
            safen = small.tile([P, 1], F32)
            nc.vector.tensor_scalar(safen[:], glob[:, 3:4], 1.0, None, OP.max)
            recn = small.tile([P, 1], F32)
            nc.vector.reciprocal(recn[:], safen[:])
            tp = small.tile([P, 1], F32)
            nc.vector.tensor_tensor(tp[:], glob[:, 4:5], recn[:], OP.mult)
            nc.vector.tensor_tensor(tp[:], tp[:], glob[:, 6:7], OP.mult)

            # ---- per-partition bisection for tau' (the k-th smallest of R') ----
            mid = small.tile([P, 1], F32)
            midt = small.tile([P, 1], F32)
            cp = small.tile([P, 1], F32)
            ge = small.tile([P, 1], F32)
            nc.vector.memset(mid[:], LO / 2)
            step = -LO / 4
            for it in range(N_ITER):
                nc.vector.tensor_scalar(junkG, G, mid[:], 0.0, OP.is_lt, OP.add,
                                        accum_out=cp[:])
                nc.vector.tensor_scalar(ge[:], cp[:], tp[:], None, OP.is_ge)
                # mid += step * (1 - 2*ge)
                nc.vector.scalar_tensor_tensor(midt[:], ge[:], -2.0 * step,
                                               mid[:], OP.mult, OP.add)
                nc.vector.tensor_scalar(mid[:], midt[:], step, None, OP.add)
                step *= 0.5
            # tau' = mean over partitions
            pt1 = psum.tile([P, 1], F32)
            nc.tensor.matmul(pt1[:], ones[:], mid[:], start=True, stop=True)
            tau = small.tile([P, 1], F32)
            nc.vector.tensor_scalar(tau[:], pt1[:], 1.0 / P, None, OP.mult)
            ntau = small.tile([P, 1], F32)
            nc.vector.tensor_scalar(ntau[:], tau[:], -1.0, None, OP.mult)

            # ---- exact pass: S' = sum(R' [R'<tau']), sgn = sum(sign(R'-tau')) ----
            sp_c = small.tile([P, N_CH], F32)
            sg_c = small.tile([P, N_CH], F32)
            for ch in range(N_CH):
                sl = slice(ch * CHUNK, (ch + 1) * CHUNK)
                nc.vector.scalar_tensor_tensor(
                    junk6[:, sl], Rp[:, sl], tau[:], Rp[:, sl], OP.is_lt,
                    OP.mult, accum_out=sp_c[:, ch:ch + 1])
                # ACT overwrites R' chunk after the DVE pass read it
                nc.scalar.activation(Rp[:, sl], Rp[:, sl], AF.Sign,
                                     bias=ntau[:], scale=1.0,
                                     accum_out=sg_c[:, ch:ch + 1])
            fin2 = small.tile([P, 4], F32)
            nc.vector.tensor_reduce(fin2[:, 0:1], sp_c[:],
                                    axis=mybir.AxisListType.X, op=OP.add)
            nc.vector.tensor_reduce(fin2[:, 1:2], sg_c[:],
                                    axis=mybir.AxisListType.X, op=OP.add)
            nc.vector.memset(fin2[:, 2:4], 0.0)

            gat2 = small.tile([P, N_CORES * 4], F32)
            _exchange(nc, tc, fin2, gat2, N_CORES, 4, rsem2, lsem, psem2,
                      expect=16, marker_eng=nc.vector)
            gv2 = gat2[:].rearrange("p (c j) -> p c j", c=N_CORES)
            fin2g = small.tile([P, 2], F32)
            nc.vector.tensor_reduce(fin2g[:, 0:1], gv2[:, :, 0],
                                    axis=mybir.AxisListType.X, op=OP.add)
            nc.vector.tensor_reduce(fin2g[:, 1:2], gv2[:, :, 1],
                                    axis=mybir.AxisListType.X, op=OP.add)
            pf = psum.tile([P, 2], F32)
            nc.tensor.matmul(pf[:], ones[:], fin2g[:], start=True, stop=True)

            # ---- final scalar assembly ----
            fin = small.tile([P, 8], F32)
            # C' = (N_total - sgn_g) / 2
            nc.vector.tensor_scalar(fin[:, 0:1], pf[:, 1:2], -0.5, N_TOTAL / 2,
                                    OP.mult, OP.add)
            # kmC = k - C'
            nc.vector.tensor_tensor(fin[:, 1:2], glob[:, 4:5], fin[:, 0:1],
                                    OP.subtract)
            # botk = S' + kmC * tau'
            nc.vector.tensor_tensor(fin[:, 2:3], fin[:, 1:2], tau[:], OP.mult)
            nc.vector.tensor_tensor(fin[:, 2:3], fin[:, 2:3], pf[:, 0:1], OP.add)
            # num = -(pos_sum' + botk)
            nc.vector.tensor_tensor(fin[:, 3:4], glob[:, 1:2], fin[:, 2:3], OP.add)
            nc.vector.tensor_scalar(fin[:, 3:4], fin[:, 3:4], -1.0, None, OP.mult)
            # den = pos_cnt + k + eps
            nc.vector.tensor_tensor(fin[:, 4:5], glob[:, 0:1], glob[:, 4:5], OP.add)
            nc.vector.tensor_scalar(fin[:, 4:5], fin[:, 4:5], EPS, None, OP.add)
            nc.vector.reciprocal(fin[:, 5:6], fin[:, 4:5])
            nc.vector.tensor_tensor(fin[:, 6:7], fin[:, 3:4], fin[:, 5:6], OP.mult)
            # debug row: loss, pos_cnt, neg_cnt, k, tau, S', C', num
            dbg = small.tile([1, 8], F32)
            nc.vector.tensor_copy(dbg[:, 0:1], fin[0:1, 6:7])
            nc.vector.tensor_copy(dbg[:, 1:2], glob[0:1, 0:1])
            nc.vector.tensor_copy(dbg[:, 2:3], glob[0:1, 3:4])
            nc.vector.tensor_copy(dbg[:, 3:4], glob[0:1, 4:5])
            nc.vector.tensor_copy(dbg[:, 4:5], tau[0:1, :])
            nc.vector.tensor_copy(dbg[:, 5:6], pf[0:1, 0:1])
            nc.vector.tensor_copy(dbg[:, 6:7], fin[0:1, 0:1])
            nc.vector.tensor_copy(dbg[:, 7:8], fin[0:1, 3:4])
            nc.sync.dma_start(out[:], dbg[:])
    nc.compile()
    return nc


def _get_nc():
    if "nc" not in _NC_CACHE:
        _NC_CACHE["nc"] = build()
    return _NC_CACHE["nc"]


def kernel(pred, gt, mask):
    pred = np.asarray(pred, dtype=np.float32)
    gt = np.asarray(gt, dtype=np.float32)
    mask = np.asarray(mask, dtype=np.float32)
    per = N // N_CORES
    in_maps = []
    for c in range(N_CORES):
        sl = slice(c * per, (c + 1) * per)
        in_maps.append({
            "pred": np.ascontiguousarray(pred[sl, 0].reshape(P, FREE)),
            "gt": np.ascontiguousarray(gt[sl, 0].reshape(P, FREE)),
            "mask": np.ascontiguousarray(mask[sl].reshape(P, FREE)),
        })
    nc = _get_nc()
    if TRACE:
        _ensure_trace_hook()
    res = run_bass_kernel_spmd(nc, in_maps, core_ids=list(range(N_CORES)),
                               trace=TRACE)
    kernel.last_result = res
    return np.float32(res.results[0]["out"][0, 0])


# revision 5
# speedup vs baseline: 82.0430x; 82.0430x over previous
"""BalanceCrossEntropyLoss on 8 trn2 NeuronCores.

Full (unsharded) inputs in, full output (scalar) out. Data-parallel over N:
each core takes 2 of the 16 images. The global top-k negative-loss sum is
computed threshold-style: a per-partition bisection on an all-gathered sample
estimates the k-th-largest threshold tau, then one exact masked sum/count pass
plus the correction  sum_topk = S(tau) + (k - C(tau)) * tau  (error is
quadratic in the tau estimation error; ~1e-5 relative here).
"""
import sys, types

sys.path.insert(0, "/opt/trn_rl_repo")
import numpy as np

import concourse.bass as bass
import concourse.bacc as bacc
import concourse.mybir as mybir
import concourse.tile as tile
from concourse.bass_utils import run_bass_kernel_spmd

F32 = mybir.dt.float32
OP = mybir.AluOpType
AF = mybir.ActivationFunctionType

N_CORES = 8
N, H, W = 16, 640, 640
P = 128                      # SBUF partitions
FREE = (N // N_CORES) * H * W // P   # 6400 columns per core
CHUNK = 1600                 # streaming chunk (4 chunks)
N_CH = FREE // CHUNK
SAMPLE_STRIDE = 64
N_SAMP = FREE // SAMPLE_STRIDE       # 100 sample columns per core
PAY = N_SAMP + 4             # AG1 payload cols: samples + pos_cnt, pos_sum', mask_sum, pad
N_TOTAL = float(N * H * W)   # 6553600 elements globally
NEG_RATIO = 3.0
EPS = 1e-6
# loss values -ln(1-p) lie in (0.01, 4.606] for p in [0.01, 0.99]; we search on
# negated values R' in [-4.75, 0]
LO = -4.75
N_ITER = 11

TRACE = False
_NC_CACHE = {}


def _ensure_trace_hook():
    import antenv
    if "antenv.axon_hooks" not in sys.modules:
        _hooks = types.ModuleType("antenv.axon_hooks")
        _hooks._hook = None
        def _set(h): _hooks._hook = h
        def _get(): return _hooks._hook
        _hooks.set_axon_ntff_profile_hook = _set
        _hooks.get_axon_ntff_profile_hook = _get
        sys.modules["antenv.axon_hooks"] = _hooks
        antenv.axon_hooks = _hooks
        from trn_agent_boot.trn_boot import _ntff_profile_via_ctypes
        _set(_ntff_profile_via_ctypes("/opt/axon/libaxon_pjrt.so"))


def build():
    nc = bacc.Bacc("TRN2", target_bir_lowering=False, debug=False,
                   num_devices=N_CORES)
    pred = nc.dram_tensor("pred", [P, FREE], F32, kind="ExternalInput").ap()
    gt = nc.dram_tensor("gt", [P, FREE], F32, kind="ExternalInput").ap()
    mask = nc.dram_tensor("mask", [P, FREE], F32, kind="ExternalInput").ap()
    out = nc.dram_tensor("out", [1, 8], F32, kind="ExternalOutput").ap()
    rg = [list(range(N_CORES))]

    with tile.TileContext(nc) as tc:
        with tc.tile_pool(name="io", bufs=2) as io, \
             tc.tile_pool(name="mids", bufs=2) as mids, \
             tc.tile_pool(name="res", bufs=1) as res, \
             tc.tile_pool(name="small", bufs=1) as small, \
             tc.tile_pool(name="psum", bufs=2, space="PSUM") as psum, \
             tc.tile_pool(name="dram", bufs=1, space="DRAM") as dram:

            # ---- warm-up collective: fires immediately (no data deps; the
            # content is irrelevant) and absorbs the ~75us first-collective
            # setup cost while streaming runs ----
            warm_in = dram.tile([P, 1], F32)
            warm_out = dram.tile([N_CORES, P, 1], F32, addr_space="Shared")
            nc.gpsimd.collective_compute(
                "AllGather", OP.bypass, replica_groups=rg,
                ins=[warm_in.opt()], outs=[warm_out.opt()])

            # ---- persistent tiles ----
            Rp = res.tile([P, FREE], F32)        # resident R' = neg * ln(1-p) <= 0
            junk6 = res.tile([P, FREE], F32)     # big scratch
            ones = small.tile([P, P], F32)
            nc.vector.memset(ones[:], 1.0)
            pcnt_c = small.tile([P, N_CH], F32)  # per-chunk accums
            psumc = small.tile([P, N_CH], F32)
            mcnt_c = small.tile([P, N_CH], F32)

            # ---- streaming phase ----
            for ch in range(N_CH):
                sl = slice(ch * CHUNK, (ch + 1) * CHUNK)
                pt = io.tile([P, CHUNK], F32, tag="pred")
                gtt = io.tile([P, CHUNK], F32, tag="gt")
                mt = io.tile([P, CHUNK], F32, tag="mask")
                nc.sync.dma_start(pt[:], pred[:, sl])
                nc.sync.dma_start(gtt[:], gt[:, sl])
                nc.sync.dma_start(mt[:], mask[:, sl])
                lp = mids.tile([P, CHUNK], F32, tag="lp")
                lq = mids.tile([P, CHUNK], F32, tag="lq")
                # ACT: ln(p), ln(1-p), and sum(mask) via Copy-accum
                nc.scalar.activation(lp[:], pt[:], AF.Ln, bias=0.0, scale=1.0)
                nc.scalar.activation(lq[:], pt[:], AF.Ln, bias=1.0, scale=-1.0)
                junka = mids.tile([P, CHUNK], F32, tag="junka")
                nc.scalar.activation(junka[:], mt[:], AF.Copy, bias=0.0,
                                     scale=1.0, accum_out=mcnt_c[:, ch:ch + 1])
                # DVE: pm = gt*mask (accum -> pos_cnt)
                pm = mids.tile([P, CHUNK], F32, tag="pm")
                nc.vector.scalar_tensor_tensor(
                    pm[:], gtt[:], 0.0, mt[:], OP.bypass, OP.mult,
                    accum_out=pcnt_c[:, ch:ch + 1])
                # GPSIMD: nm = mask - pm
                nm = mids.tile([P, CHUNK], F32, tag="nm")
                nc.gpsimd.tensor_tensor(nm[:], mt[:], pm[:], OP.subtract)
                # DVE: R' = lq * nm  (resident)
                nc.vector.scalar_tensor_tensor(
                    Rp[:, sl], lq[:], 0.0, nm[:], OP.bypass, OP.mult)
                # DVE: pos-loss partial: (lp)*pm, accum -> pos_sum' (= -pos_sum)
                junkb = mids.tile([P, CHUNK], F32, tag="junkb")
                nc.vector.scalar_tensor_tensor(
                    junkb[:], lp[:], 0.0, pm[:], OP.bypass, OP.mult,
                    accum_out=psumc[:, ch:ch + 1])

            # ---- reduce per-chunk accums, pack AG1 payload ----
            pay = small.tile([P, PAY], F32)
            # sample: every 64th column of R'
            samp_view = Rp[:].rearrange("p (n s) -> p n s", s=SAMPLE_STRIDE)[:, :, 0]
            nc.vector.tensor_copy(pay[:, 0:N_SAMP], samp_view)
            nc.vector.tensor_reduce(pay[:, N_SAMP:N_SAMP + 1], pcnt_c[:],
                                    axis=mybir.AxisListType.X, op=OP.add)
            nc.vector.tensor_reduce(pay[:, N_SAMP + 1:N_SAMP + 2], psumc[:],
                                    axis=mybir.AxisListType.X, op=OP.add)
            nc.vector.tensor_reduce(pay[:, N_SAMP + 2:N_SAMP + 3], mcnt_c[:],
                                    axis=mybir.AxisListType.X, op=OP.add)
            nc.vector.memset(pay[:, N_SAMP + 3:N_SAMP + 4], 0.0)

            ag1_in = dram.tile([P, PAY], F32)
            ag1_out = dram.tile([N_CORES, P, PAY], F32, addr_space="Shared")
            nc.sync.dma_start(ag1_in[:], pay[:])
            nc.gpsimd.collective_compute(
                "AllGather", OP.bypass, replica_groups=rg,
                ins=[ag1_in.opt()], outs=[ag1_out.opt()])
            gat = small.tile([P, N_CORES * PAY], F32)
            nc.sync.dma_start(
                gat[:].rearrange("p (c j) -> p c j", c=N_CORES),
                ag1_out[:].rearrange("c p j -> p c j"))

            # views into the gathered payload
            gv = gat[:].rearrange("p (c j) -> p c j", c=N_CORES)
            G = gv[:, :, 0:N_SAMP]                 # [P, 8, 100] samples
            junkG = junk6[:, 0:N_CORES * PAY].rearrange(
                "p (c j) -> p c j", c=N_CORES)[:, :, 0:N_SAMP]

            # ---- global exact counts: pos_cnt, pos_sum', mask_sum ----
            st3 = small.tile([P, 3], F32)
            for i in range(3):
                nc.vector.tensor_reduce(st3[:, i:i + 1], gv[:, :, N_SAMP + i],
                                        axis=mybir.AxisListType.X, op=OP.add)
            ps3 = psum.tile([P, 3], F32)
            nc.tensor.matmul(ps3[:], ones[:], st3[:], start=True, stop=True)
            glob = small.tile([P, 8], F32)  # 0:pos_cnt 1:pos_sum' 2:mask_sum 3:neg_cnt 4:k 5:t_p 6:c0 7:scratch
            nc.vector.tensor_copy(glob[:, 0:3], ps3[:])
            # neg_cnt = mask_sum - pos_cnt
            nc.vector.tensor_tensor(glob[:, 3:4], glob[:, 2:3], glob[:, 0:1],
                                    OP.subtract)
            # k = min(neg_cnt, 3*pos_cnt)
            nc.vector.tensor_scalar(glob[:, 7:8], glob[:, 0:1], NEG_RATIO, None,
                                    OP.mult)
            nc.vector.tensor_tensor(glob[:, 4:5], glob[:, 3:4], glob[:, 7:8],
                                    OP.min)
            # c0_p = per-partition count of valid (nonzero) sampled negatives
            nc.vector.tensor_scalar(junkG, G, -1e-3, 0.0, OP.is_lt, OP.add,
                                    accum_out=glob[:, 6:7])
            # t_p = k * c0_p / neg_cnt
            safen = small.tile([P, 1], F32)
            nc.vector.tensor_scalar(safen[:], glob[:, 3:4], 1.0, None, OP.max)
            recn = small.tile([P, 1], F32)
            nc.vector.reciprocal(recn[:], safen[:])
            tp = small.tile([P, 1], F32)
            nc.vector.tensor_tensor(tp[:], glob[:, 4:5], recn[:], OP.mult)
            nc.vector.tensor_tensor(tp[:], tp[:], glob[:, 6:7], OP.mult)

            # ---- per-partition bisection for tau' (the k-th smallest of R') ----
            mid = small.tile([P, 1], F32)
            midt = small.tile([P, 1], F32)
            cp = small.tile([P, 1], F32)
            ge = small.tile([P, 1], F32)
            nc.vector.memset(mid[:], LO / 2)
            step = -LO / 4
            for it in range(N_ITER):
                nc.vector.tensor_scalar(junkG, G, mid[:], 0.0, OP.is_lt, OP.add,
                                        accum_out=cp[:])
                nc.vector.tensor_scalar(ge[:], cp[:], tp[:], None, OP.is_ge)
                # mid += step * (1 - 2*ge)
                nc.vector.scalar_tensor_tensor(midt[:], ge[:], -2.0 * step,
                                               mid[:], OP.mult, OP.add)
                nc.vector.tensor_scalar(mid[:], midt[:], step, None, OP.add)
                step *= 0.5
            # tau' = mean over partitions
            pt1 = psum.tile([P, 1], F32)
            nc.tensor.matmul(pt1[:], ones[:], mid[:], start=True, stop=True)
            tau = small.tile([P, 1], F32)
            nc.vector.tensor_scalar(tau[:], pt1[:], 1.0 / P, None, OP.mult)
            ntau = small.tile([P, 1], F32)
            nc.vector.tensor_scalar(ntau[:], tau[:], -1.0, None, OP.mult)

            # ---- exact pass: S' = sum(R' [R'<tau']), sgn = sum(sign(R'-tau')) ----
            sp_c = small.tile([P, N_CH], F32)
            sg_c = small.tile([P, N_CH], F32)
            for ch in range(N_CH):
                sl = slice(ch * CHUNK, (ch + 1) * CHUNK)
                nc.vector.scalar_tensor_tensor(
                    junk6[:, sl], Rp[:, sl], tau[:], Rp[:, sl], OP.is_lt,
                    OP.mult, accum_out=sp_c[:, ch:ch + 1])
                # ACT overwrites R' chunk after the DVE pass read it
                nc.scalar.activation(Rp[:, sl], Rp[:, sl], AF.Sign,
                                     bias=ntau[:], scale=1.0,
                                     accum_out=sg_c[:, ch:ch + 1])
            fin2 = small.tile([P, 2], F32)
            nc.vector.tensor_reduce(fin2[:, 0:1], sp_c[:],
                                    axis=mybir.AxisListType.X, op=OP.add)
            nc.vector.tensor_reduce(fin2[:, 1:2], sg_c[:],
                                    axis=mybir.AxisListType.X, op=OP.add)

            ag2_in = dram.tile([P, 2], F32)
            ag2_out = dram.tile([N_CORES, P, 2], F32, addr_space="Shared")
            nc.sync.dma_start(ag2_in[:], fin2[:])
            nc.gpsimd.collective_compute(
                "AllGather", OP.bypass, replica_groups=rg,
                ins=[ag2_in.opt()], outs=[ag2_out.opt()])
            gat2 = small.tile([P, N_CORES * 2], F32)
            nc.sync.dma_start(
                gat2[:].rearrange("p (c j) -> p c j", c=N_CORES),
                ag2_out[:].rearrange("c p j -> p c j"))
            gv2 = gat2[:].rearrange("p (c j) -> p c j", c=N_CORES)
            fin2g = small.tile([P, 2], F32)
            nc.vector.tensor_reduce(fin2g[:, 0:1], gv2[:, :, 0],
                                    axis=mybir.AxisListType.X, op=OP.add)
            nc.vector.tensor_reduce(fin2g[:, 1:2], gv2[:, :, 1],
                                    axis=mybir.AxisListType.X, op=OP.add)
            pf = psum.tile([P, 2], F32)
            nc.tensor.matmul(pf[:], ones[:], fin2g[:], start=True, stop=True)

            # ---- final scalar assembly ----
            fin = small.tile([P, 8], F32)
            # C' = (N_total - sgn_g) / 2
            nc.vector.tensor_scalar(fin[:, 0:1], pf[:, 1:2], -0.5, N_TOTAL / 2,
                                    OP.mult, OP.add)
            # kmC = k - C'
            nc.vector.tensor_tensor(fin[:, 1:2], glob[:, 4:5], fin[:, 0:1],
                                    OP.subtract)
            # botk = S' + kmC * tau'
            nc.vector.tensor_tensor(fin[:, 2:3], fin[:, 1:2], tau[:], OP.mult)
            nc.vector.tensor_tensor(fin[:, 2:3], fin[:, 2:3], pf[:, 0:1], OP.add)
            # num = -(pos_sum' + botk)
            nc.vector.tensor_tensor(fin[:, 3:4], glob[:, 1:2], fin[:, 2:3], OP.add)
            nc.vector.tensor_scalar(fin[:, 3:4], fin[:, 3:4], -1.0, None, OP.mult)
            # den = pos_cnt + k + eps
            nc.vector.tensor_tensor(fin[:, 4:5], glob[:, 0:1], glob[:, 4:5], OP.add)
            nc.vector.tensor_scalar(fin[:, 4:5], fin[:, 4:5], EPS, None, OP.add)
            nc.vector.reciprocal(fin[:, 5:6], fin[:, 4:5])
            nc.vector.tensor_tensor(fin[:, 6:7], fin[:, 3:4], fin[:, 5:6], OP.mult)
            # debug row: loss, pos_cnt, neg_cnt, k, tau, S', C', num
            dbg = small.tile([1, 8], F32)
            nc.vector.tensor_copy(dbg[:, 0:1], fin[0:1, 6:7])
            nc.vector.tensor_copy(dbg[:, 1:2], glob[0:1, 0:1])
            nc.vector.tensor_copy(dbg[:, 2:3], glob[0:1, 3:4])
            nc.vector.tensor_copy(dbg[:, 3:4], glob[0:1, 4:5])
            nc.vector.tensor_copy(dbg[:, 4:5], tau[0:1, :])
            nc.vector.tensor_copy(dbg[:, 5:6], pf[0:1, 0:1])
            nc.vector.tensor_copy(dbg[:, 6:7], fin[0:1, 0:1])
            nc.vector.tensor_copy(dbg[:, 7:8], fin[0:1, 3:4])
            nc.sync.dma_start(out[:], dbg[:])
    nc.compile()
    return nc


def _get_nc():
    if "nc" not in _NC_CACHE:
        _NC_CACHE["nc"] = build()
    return _NC_CACHE["nc"]


def kernel(pred, gt, mask):
    pred = np.asarray(pred, dtype=np.float32)
    gt = np.asarray(gt, dtype=np.float32)
    mask = np.asarray(mask, dtype=np.float32)
    per = N // N_CORES
    in_maps = []
    for c in range(N_CORES):
        sl = slice(c * per, (c + 1) * per)
        in_maps.append({
            "pred": np.ascontiguousarray(pred[sl, 0].reshape(P, FREE)),
            "gt": np.ascontiguousarray(gt[sl, 0].reshape(P, FREE)),
            "mask": np.ascontiguousarray(mask[sl].reshape(P, FREE)),
        })
    nc = _get_nc()
    if TRACE:
        _ensure_trace_hook()
    res = run_bass_kernel_spmd(nc, in_maps, core_ids=list(range(N_CORES)),
                               trace=TRACE)
    kernel.last_result = res
    return np.float32(res.results[0]["out"][0, 0])


# revision 6
# speedup vs baseline: 86.8807x; 1.0590x over previous
"""BalanceCrossEntropyLoss on 8 trn2 NeuronCores.

Full (unsharded) inputs in, full output (scalar) out. Data-parallel over N:
each core takes 2 of the 16 images. The global top-k negative-loss sum is
computed threshold-style: a per-partition bisection on an all-gathered sample
estimates the k-th-largest threshold tau, then one exact masked sum/count pass
plus the correction  sum_topk = S(tau) + (k - C(tau)) * tau  (error is
quadratic in the tau estimation error; ~1e-5 relative here).
"""
import sys, types

sys.path.insert(0, "/opt/trn_rl_repo")
import numpy as np

import concourse.bass as bass
import concourse.bacc as bacc
import concourse.mybir as mybir
import concourse.tile as tile
from concourse.bass_utils import run_bass_kernel_spmd

F32 = mybir.dt.float32
OP = mybir.AluOpType
AF = mybir.ActivationFunctionType

N_CORES = 8
N, H, W = 16, 640, 640
P = 128                      # SBUF partitions
FREE = (N // N_CORES) * H * W // P   # 6400 columns per core
CHUNK = 1600                 # streaming chunk (4 chunks)
N_CH = FREE // CHUNK
SAMPLE_STRIDE = 64
N_SAMP = FREE // SAMPLE_STRIDE       # 100 sample columns per core
PAY = N_SAMP + 4             # AG1 payload cols: samples + pos_cnt, pos_sum', mask_sum, pad
N_TOTAL = float(N * H * W)   # 6553600 elements globally
NEG_RATIO = 3.0
EPS = 1e-6
# loss values -ln(1-p) lie in (0.01, 4.606] for p in [0.01, 0.99]; we search on
# negated values R' in [-4.75, 0]
LO = -4.75
N_ITER = 11
N_REFINE = 5

TRACE = False
_NC_CACHE = {}


def _ensure_trace_hook():
    import antenv
    if "antenv.axon_hooks" not in sys.modules:
        _hooks = types.ModuleType("antenv.axon_hooks")
        _hooks._hook = None
        def _set(h): _hooks._hook = h
        def _get(): return _hooks._hook
        _hooks.set_axon_ntff_profile_hook = _set
        _hooks.get_axon_ntff_profile_hook = _get
        sys.modules["antenv.axon_hooks"] = _hooks
        antenv.axon_hooks = _hooks
        from trn_agent_boot.trn_boot import _ntff_profile_via_ctypes
        _set(_ntff_profile_via_ctypes("/opt/axon/libaxon_pjrt.so"))


def build():
    nc = bacc.Bacc("TRN2", target_bir_lowering=False, debug=False,
                   num_devices=N_CORES)
    pred = nc.dram_tensor("pred", [P, FREE], F32, kind="ExternalInput").ap()
    gt = nc.dram_tensor("gt", [P, FREE], F32, kind="ExternalInput").ap()
    mask = nc.dram_tensor("mask", [P, FREE], F32, kind="ExternalInput").ap()
    out = nc.dram_tensor("out", [1, 8], F32, kind="ExternalOutput").ap()
    rg = [list(range(N_CORES))]

    with tile.TileContext(nc) as tc:
        with tc.tile_pool(name="io", bufs=2) as io, \
             tc.tile_pool(name="mids", bufs=2) as mids, \
             tc.tile_pool(name="res", bufs=1) as res, \
             tc.tile_pool(name="small", bufs=1) as small, \
             tc.tile_pool(name="psum", bufs=2, space="PSUM") as psum, \
             tc.tile_pool(name="dram", bufs=1, space="DRAM") as dram:

            # ---- warm-up collective: fires immediately (no data deps; the
            # content is irrelevant) and absorbs the ~75us first-collective
            # setup cost while streaming runs ----
            warm_in = dram.tile([P, 1], F32)
            warm_out = dram.tile([N_CORES, P, 1], F32, addr_space="Shared")
            nc.gpsimd.collective_compute(
                "AllGather", OP.bypass, replica_groups=rg,
                ins=[warm_in.opt()], outs=[warm_out.opt()])

            # ---- persistent tiles ----
            Rp = res.tile([P, FREE], F32)        # resident R' = neg * ln(1-p) <= 0
            junk6 = res.tile([P, FREE], F32)     # big scratch
            ones = small.tile([P, P], F32)
            nc.vector.memset(ones[:], 1.0)
            pcnt_c = small.tile([P, N_CH], F32)  # per-chunk accums
            psumc = small.tile([P, N_CH], F32)
            mcnt_c = small.tile([P, N_CH], F32)

            # ---- streaming phase ----
            for ch in range(N_CH):
                sl = slice(ch * CHUNK, (ch + 1) * CHUNK)
                pt = io.tile([P, CHUNK], F32, tag="pred")
                gtt = io.tile([P, CHUNK], F32, tag="gt")
                mt = io.tile([P, CHUNK], F32, tag="mask")
                nc.sync.dma_start(pt[:], pred[:, sl])
                nc.sync.dma_start(gtt[:], gt[:, sl])
                nc.sync.dma_start(mt[:], mask[:, sl])
                lp = mids.tile([P, CHUNK], F32, tag="lp")
                lq = mids.tile([P, CHUNK], F32, tag="lq")
                # ACT: ln(p), ln(1-p), and sum(mask) via Copy-accum
                nc.scalar.activation(lp[:], pt[:], AF.Ln, bias=0.0, scale=1.0)
                nc.scalar.activation(lq[:], pt[:], AF.Ln, bias=1.0, scale=-1.0)
                junka = mids.tile([P, CHUNK], F32, tag="junka")
                nc.scalar.activation(junka[:], mt[:], AF.Copy, bias=0.0,
                                     scale=1.0, accum_out=mcnt_c[:, ch:ch + 1])
                # DVE: pm = gt*mask (accum -> pos_cnt)
                pm = mids.tile([P, CHUNK], F32, tag="pm")
                nc.vector.scalar_tensor_tensor(
                    pm[:], gtt[:], 0.0, mt[:], OP.bypass, OP.mult,
                    accum_out=pcnt_c[:, ch:ch + 1])
                # GPSIMD: nm = mask - pm
                nm = mids.tile([P, CHUNK], F32, tag="nm")
                nc.gpsimd.tensor_tensor(nm[:], mt[:], pm[:], OP.subtract)
                # DVE: R' = lq * nm  (resident)
                nc.vector.scalar_tensor_tensor(
                    Rp[:, sl], lq[:], 0.0, nm[:], OP.bypass, OP.mult)
                # DVE: pos-loss partial: (lp)*pm, accum -> pos_sum' (= -pos_sum)
                junkb = mids.tile([P, CHUNK], F32, tag="junkb")
                nc.vector.scalar_tensor_tensor(
                    junkb[:], lp[:], 0.0, pm[:], OP.bypass, OP.mult,
                    accum_out=psumc[:, ch:ch + 1])

            # ---- reduce per-chunk accums, pack AG1 payload ----
            pay = small.tile([P, PAY], F32)
            # sample: every 64th column of R'
            samp_view = Rp[:].rearrange("p (n s) -> p n s", s=SAMPLE_STRIDE)[:, :, 0]
            nc.vector.tensor_copy(pay[:, 0:N_SAMP], samp_view)
            nc.vector.tensor_reduce(pay[:, N_SAMP:N_SAMP + 1], pcnt_c[:],
                                    axis=mybir.AxisListType.X, op=OP.add)
            nc.vector.tensor_reduce(pay[:, N_SAMP + 1:N_SAMP + 2], psumc[:],
                                    axis=mybir.AxisListType.X, op=OP.add)
            nc.vector.tensor_reduce(pay[:, N_SAMP + 2:N_SAMP + 3], mcnt_c[:],
                                    axis=mybir.AxisListType.X, op=OP.add)
            nc.vector.memset(pay[:, N_SAMP + 3:N_SAMP + 4], 0.0)

            # ---- local pre-search on own sample: runs in the dead window
            # while the warm-up collective's ncfw setup (~70us) completes ----
            mid = small.tile([P, 1], F32)
            midt = small.tile([P, 1], F32)
            cp = small.tile([P, 1], F32)
            ge = small.tile([P, 1], F32)
            locg = small.tile([P, 8], F32)  # 0:neg_l 1:k_l 2:t_l 3:c0_l
            junkL = junk6[:, 0:N_SAMP]
            Gl = pay[:, 0:N_SAMP]
            nc.vector.tensor_tensor(locg[:, 0:1], pay[:, N_SAMP + 2:N_SAMP + 3],
                                    pay[:, N_SAMP:N_SAMP + 1], OP.subtract)
            nc.vector.tensor_scalar(locg[:, 4:5], pay[:, N_SAMP:N_SAMP + 1],
                                    NEG_RATIO, None, OP.mult)
            nc.vector.tensor_tensor(locg[:, 1:2], locg[:, 0:1], locg[:, 4:5],
                                    OP.min)
            nc.vector.tensor_scalar(junkL, Gl, -1e-3, 0.0, OP.is_lt, OP.add,
                                    accum_out=locg[:, 3:4])
            nc.vector.tensor_scalar(locg[:, 5:6], locg[:, 0:1], 1.0, None, OP.max)
            locrec = small.tile([P, 1], F32)
            nc.vector.reciprocal(locrec[:], locg[:, 5:6])
            nc.vector.tensor_tensor(locg[:, 2:3], locg[:, 1:2], locrec[:], OP.mult)
            nc.vector.tensor_tensor(locg[:, 2:3], locg[:, 2:3], locg[:, 3:4],
                                    OP.mult)
            nc.vector.memset(mid[:], LO / 2)
            step = -LO / 4
            for it in range(N_ITER):
                nc.vector.tensor_scalar(junkL, Gl, mid[:], 0.0, OP.is_lt, OP.add,
                                        accum_out=cp[:])
                nc.vector.tensor_scalar(ge[:], cp[:], locg[:, 2:3], None, OP.is_ge)
                nc.vector.scalar_tensor_tensor(midt[:], ge[:], -2.0 * step,
                                               mid[:], OP.mult, OP.add)
                nc.vector.tensor_scalar(mid[:], midt[:], step, None, OP.add)
                step *= 0.5
            # tau0 = mean over partitions of the local estimates
            pt0 = psum.tile([P, 1], F32)
            nc.tensor.matmul(pt0[:], ones[:], mid[:], start=True, stop=True)
            tau0 = small.tile([P, 1], F32)
            nc.vector.tensor_scalar(tau0[:], pt0[:], 1.0 / P, None, OP.mult)

            ag1_in = dram.tile([P, PAY], F32)
            ag1_out = dram.tile([N_CORES, P, PAY], F32, addr_space="Shared")
            nc.sync.dma_start(ag1_in[:], pay[:])
            nc.gpsimd.collective_compute(
                "AllGather", OP.bypass, replica_groups=rg,
                ins=[ag1_in.opt()], outs=[ag1_out.opt()])
            gat = small.tile([P, N_CORES * PAY], F32)
            nc.sync.dma_start(
                gat[:].rearrange("p (c j) -> p c j", c=N_CORES),
                ag1_out[:].rearrange("c p j -> p c j"))

            # views into the gathered payload
            gv = gat[:].rearrange("p (c j) -> p c j", c=N_CORES)
            G = gv[:, :, 0:N_SAMP]                 # [P, 8, 100] samples
            junkG = junk6[:, 0:N_CORES * PAY].rearrange(
                "p (c j) -> p c j", c=N_CORES)[:, :, 0:N_SAMP]

            # ---- global exact counts: pos_cnt, pos_sum', mask_sum ----
            st3 = small.tile([P, 3], F32)
            for i in range(3):
                nc.vector.tensor_reduce(st3[:, i:i + 1], gv[:, :, N_SAMP + i],
                                        axis=mybir.AxisListType.X, op=OP.add)
            ps3 = psum.tile([P, 3], F32)
            nc.tensor.matmul(ps3[:], ones[:], st3[:], start=True, stop=True)
            glob = small.tile([P, 8], F32)  # 0:pos_cnt 1:pos_sum' 2:mask_sum 3:neg_cnt 4:k 5:t_p 6:c0 7:scratch
            nc.vector.tensor_copy(glob[:, 0:3], ps3[:])
            # neg_cnt = mask_sum - pos_cnt
            nc.vector.tensor_tensor(glob[:, 3:4], glob[:, 2:3], glob[:, 0:1],
                                    OP.subtract)
            # k = min(neg_cnt, 3*pos_cnt)
            nc.vector.tensor_scalar(glob[:, 7:8], glob[:, 0:1], NEG_RATIO, None,
                                    OP.mult)
            nc.vector.tensor_tensor(glob[:, 4:5], glob[:, 3:4], glob[:, 7:8],
                                    OP.min)
            # c0_p = per-partition count of valid (nonzero) sampled negatives
            nc.vector.tensor_scalar(junkG, G, -1e-3, 0.0, OP.is_lt, OP.add,
                                    accum_out=glob[:, 6:7])
            # t_p = k * c0_p / neg_cnt
            safen = small.tile([P, 1], F32)
            nc.vector.tensor_scalar(safen[:], glob[:, 3:4], 1.0, None, OP.max)
            recn = small.tile([P, 1], F32)
            nc.vector.reciprocal(recn[:], safen[:])
            tp = small.tile([P, 1], F32)
            nc.vector.tensor_tensor(tp[:], glob[:, 4:5], recn[:], OP.mult)
            nc.vector.tensor_tensor(tp[:], tp[:], glob[:, 6:7], OP.mult)

            # ---- global refinement from tau0 on the gathered sample ----
            nc.vector.tensor_copy(mid[:], tau0[:])
            step = 0.04
            for it in range(N_REFINE):
                nc.vector.tensor_scalar(junkG, G, mid[:], 0.0, OP.is_lt, OP.add,
                                        accum_out=cp[:])
                nc.vector.tensor_scalar(ge[:], cp[:], tp[:], None, OP.is_ge)
                # mid += step * (1 - 2*ge)
                nc.vector.scalar_tensor_tensor(midt[:], ge[:], -2.0 * step,
                                               mid[:], OP.mult, OP.add)
                nc.vector.tensor_scalar(mid[:], midt[:], step, None, OP.add)
                step *= 0.5
            # tau' = mean over partitions
            pt1 = psum.tile([P, 1], F32)
            nc.tensor.matmul(pt1[:], ones[:], mid[:], start=True, stop=True)
            tau = small.tile([P, 1], F32)
            nc.vector.tensor_scalar(tau[:], pt1[:], 1.0 / P, None, OP.mult)
            ntau = small.tile([P, 1], F32)
            nc.vector.tensor_scalar(ntau[:], tau[:], -1.0, None, OP.mult)

            # ---- exact pass: S' = sum(R' [R'<tau']), sgn = sum(sign(R'-tau')) ----
            sp_c = small.tile([P, N_CH], F32)
            sg_c = small.tile([P, N_CH], F32)
            for ch in range(N_CH):
                sl = slice(ch * CHUNK, (ch + 1) * CHUNK)
                nc.vector.scalar_tensor_tensor(
                    junk6[:, sl], Rp[:, sl], tau[:], Rp[:, sl], OP.is_lt,
                    OP.mult, accum_out=sp_c[:, ch:ch + 1])
                # ACT overwrites R' chunk after the DVE pass read it
                nc.scalar.activation(Rp[:, sl], Rp[:, sl], AF.Sign,
                                     bias=ntau[:], scale=1.0,
                                     accum_out=sg_c[:, ch:ch + 1])
            fin2 = small.tile([P, 2], F32)
            nc.vector.tensor_reduce(fin2[:, 0:1], sp_c[:],
                                    axis=mybir.AxisListType.X, op=OP.add)
            nc.vector.tensor_reduce(fin2[:, 1:2], sg_c[:],
                                    axis=mybir.AxisListType.X, op=OP.add)

            ag2_in = dram.tile([P, 2], F32)
            ag2_out = dram.tile([N_CORES, P, 2], F32, addr_space="Shared")
            nc.sync.dma_start(ag2_in[:], fin2[:])
            nc.gpsimd.collective_compute(
                "AllGather", OP.bypass, replica_groups=rg,
                ins=[ag2_in.opt()], outs=[ag2_out.opt()])
            gat2 = small.tile([P, N_CORES * 2], F32)
            nc.sync.dma_start(
                gat2[:].rearrange("p (c j) -> p c j", c=N_CORES),
                ag2_out[:].rearrange("c p j -> p c j"))
            gv2 = gat2[:].rearrange("p (c j) -> p c j", c=N_CORES)
            fin2g = small.tile([P, 2], F32)
            nc.vector.tensor_reduce(fin2g[:, 0:1], gv2[:, :, 0],
                                    axis=mybir.AxisListType.X, op=OP.add)
            nc.vector.tensor_reduce(fin2g[:, 1:2], gv2[:, :, 1],
                                    axis=mybir.AxisListType.X, op=OP.add)
            pf = psum.tile([P, 2], F32)
            nc.tensor.matmul(pf[:], ones[:], fin2g[:], start=True, stop=True)

            # ---- final scalar assembly ----
            fin = small.tile([P, 8], F32)
            # C' = (N_total - sgn_g) / 2
            nc.vector.tensor_scalar(fin[:, 0:1], pf[:, 1:2], -0.5, N_TOTAL / 2,
                                    OP.mult, OP.add)
            # kmC = k - C'
            nc.vector.tensor_tensor(fin[:, 1:2], glob[:, 4:5], fin[:, 0:1],
                                    OP.subtract)
            # botk = S' + kmC * tau'
            nc.vector.tensor_tensor(fin[:, 2:3], fin[:, 1:2], tau[:], OP.mult)
            nc.vector.tensor_tensor(fin[:, 2:3], fin[:, 2:3], pf[:, 0:1], OP.add)
            # num = -(pos_sum' + botk)
            nc.vector.tensor_tensor(fin[:, 3:4], glob[:, 1:2], fin[:, 2:3], OP.add)
            nc.vector.tensor_scalar(fin[:, 3:4], fin[:, 3:4], -1.0, None, OP.mult)
            # den = pos_cnt + k + eps
            nc.vector.tensor_tensor(fin[:, 4:5], glob[:, 0:1], glob[:, 4:5], OP.add)
            nc.vector.tensor_scalar(fin[:, 4:5], fin[:, 4:5], EPS, None, OP.add)
            nc.vector.reciprocal(fin[:, 5:6], fin[:, 4:5])
            nc.vector.tensor_tensor(fin[:, 6:7], fin[:, 3:4], fin[:, 5:6], OP.mult)
            # debug row: loss, pos_cnt, neg_cnt, k, tau, S', C', num
            dbg = small.tile([1, 8], F32)
            nc.vector.tensor_copy(dbg[:, 0:1], fin[0:1, 6:7])
            nc.vector.tensor_copy(dbg[:, 1:2], glob[0:1, 0:1])
            nc.vector.tensor_copy(dbg[:, 2:3], glob[0:1, 3:4])
            nc.vector.tensor_copy(dbg[:, 3:4], glob[0:1, 4:5])
            nc.vector.tensor_copy(dbg[:, 4:5], tau[0:1, :])
            nc.vector.tensor_copy(dbg[:, 5:6], pf[0:1, 0:1])
            nc.vector.tensor_copy(dbg[:, 6:7], fin[0:1, 0:1])
            nc.vector.tensor_copy(dbg[:, 7:8], fin[0:1, 3:4])
            nc.sync.dma_start(out[:], dbg[:])
    nc.compile()
    return nc


def _get_nc():
    if "nc" not in _NC_CACHE:
        _NC_CACHE["nc"] = build()
    return _NC_CACHE["nc"]


def kernel(pred, gt, mask):
    pred = np.asarray(pred, dtype=np.float32)
    gt = np.asarray(gt, dtype=np.float32)
    mask = np.asarray(mask, dtype=np.float32)
    per = N // N_CORES
    in_maps = []
    for c in range(N_CORES):
        sl = slice(c * per, (c + 1) * per)
        in_maps.append({
            "pred": np.ascontiguousarray(pred[sl, 0].reshape(P, FREE)),
            "gt": np.ascontiguousarray(gt[sl, 0].reshape(P, FREE)),
            "mask": np.ascontiguousarray(mask[sl].reshape(P, FREE)),
        })
    nc = _get_nc()
    if TRACE:
        _ensure_trace_hook()
    res = run_bass_kernel_spmd(nc, in_maps, core_ids=list(range(N_CORES)),
                               trace=TRACE)
    kernel.last_result = res
    return np.float32(res.results[0]["out"][0, 0])


# revision 7
# speedup vs baseline: 93.2141x; 1.0729x over previous
"""BalanceCrossEntropyLoss on 8 trn2 NeuronCores.

Full (unsharded) inputs in, full output (scalar) out. Data-parallel over N:
each core takes 2 of the 16 images. The global top-k negative-loss sum is
computed threshold-style: a per-partition bisection on an all-gathered sample
estimates the k-th-largest threshold tau, then one exact masked sum/count pass
plus the correction  sum_topk = S(tau) + (k - C(tau)) * tau  (error is
quadratic in the tau estimation error; ~1e-5 relative here).
"""
import sys, types

sys.path.insert(0, "/opt/trn_rl_repo")
import numpy as np

import concourse.bass as bass
import concourse.bacc as bacc
import concourse.mybir as mybir
import concourse.tile as tile
from concourse.bass_utils import run_bass_kernel_spmd

F32 = mybir.dt.float32
OP = mybir.AluOpType
AF = mybir.ActivationFunctionType

N_CORES = 8
N, H, W = 16, 640, 640
P = 128                      # SBUF partitions
FREE = (N // N_CORES) * H * W // P   # 6400 columns per core
CHUNK = 1600                 # streaming chunk (4 chunks)
N_CH = FREE // CHUNK
SAMPLE_STRIDE = 64
N_SAMP = FREE // SAMPLE_STRIDE       # 100 sample columns per core
PAY = N_SAMP + 4             # AG1 payload cols: samples + pos_cnt, pos_sum', mask_sum, pad
N_TOTAL = float(N * H * W)   # 6553600 elements globally
NEG_RATIO = 3.0
EPS = 1e-6
# loss values -ln(1-p) lie in (0.01, 4.606] for p in [0.01, 0.99]; we search on
# negated values R' in [-4.75, 0]
LO = -4.75
N_ITER = 11
N_REFINE = 4

TRACE = False
_NC_CACHE = {}


def _ensure_trace_hook():
    import antenv
    if "antenv.axon_hooks" not in sys.modules:
        _hooks = types.ModuleType("antenv.axon_hooks")
        _hooks._hook = None
        def _set(h): _hooks._hook = h
        def _get(): return _hooks._hook
        _hooks.set_axon_ntff_profile_hook = _set
        _hooks.get_axon_ntff_profile_hook = _get
        sys.modules["antenv.axon_hooks"] = _hooks
        antenv.axon_hooks = _hooks
        from trn_agent_boot.trn_boot import _ntff_profile_via_ctypes
        _set(_ntff_profile_via_ctypes("/opt/axon/libaxon_pjrt.so"))


def build():
    nc = bacc.Bacc("TRN2", target_bir_lowering=False, debug=False,
                   num_devices=N_CORES)
    pred = nc.dram_tensor("pred", [P, FREE], F32, kind="ExternalInput").ap()
    gt = nc.dram_tensor("gt", [P, FREE], F32, kind="ExternalInput").ap()
    mask = nc.dram_tensor("mask", [P, FREE], F32, kind="ExternalInput").ap()
    out = nc.dram_tensor("out", [1, 8], F32, kind="ExternalOutput").ap()
    rg = [list(range(N_CORES))]

    with tile.TileContext(nc) as tc:
        with tc.tile_pool(name="io", bufs=2) as io, \
             tc.tile_pool(name="mids", bufs=2) as mids, \
             tc.tile_pool(name="res", bufs=1) as res, \
             tc.tile_pool(name="small", bufs=1) as small, \
             tc.tile_pool(name="psum", bufs=2, space="PSUM") as psum, \
             tc.tile_pool(name="dram", bufs=1, space="DRAM") as dram:

            # ---- warm-up collective: fires immediately (no data deps; the
            # content is irrelevant) and absorbs the ~75us first-collective
            # setup cost while streaming runs ----
            warm_in = dram.tile([P, 1], F32)
            warm_out = dram.tile([N_CORES, P, 1], F32, addr_space="Shared")
            nc.gpsimd.collective_compute(
                "AllGather", OP.bypass, replica_groups=rg,
                ins=[warm_in.opt()], outs=[warm_out.opt()])

            # ---- persistent tiles ----
            Rp = res.tile([P, FREE], F32)        # resident R' = neg * ln(1-p) <= 0
            junk6 = res.tile([P, FREE], F32)     # big scratch
            ones = small.tile([P, P], F32)
            nc.vector.memset(ones[:], 1.0)
            pcnt_c = small.tile([P, N_CH], F32)  # per-chunk accums
            psumc = small.tile([P, N_CH], F32)
            mcnt_c = small.tile([P, N_CH], F32)

            # ---- streaming phase ----
            for ch in range(N_CH):
                sl = slice(ch * CHUNK, (ch + 1) * CHUNK)
                pt = io.tile([P, CHUNK], F32, tag="pred")
                gtt = io.tile([P, CHUNK], F32, tag="gt")
                mt = io.tile([P, CHUNK], F32, tag="mask")
                nc.sync.dma_start(pt[:], pred[:, sl])
                nc.sync.dma_start(gtt[:], gt[:, sl])
                nc.sync.dma_start(mt[:], mask[:, sl])
                lp = mids.tile([P, CHUNK], F32, tag="lp")
                lq = mids.tile([P, CHUNK], F32, tag="lq")
                # ACT: ln(p), ln(1-p), and sum(mask) via Copy-accum
                nc.scalar.activation(lp[:], pt[:], AF.Ln, bias=0.0, scale=1.0)
                nc.scalar.activation(lq[:], pt[:], AF.Ln, bias=1.0, scale=-1.0)
                junka = mids.tile([P, CHUNK], F32, tag="junka")
                nc.scalar.activation(junka[:], mt[:], AF.Copy, bias=0.0,
                                     scale=1.0, accum_out=mcnt_c[:, ch:ch + 1])
                # DVE: pm = gt*mask (accum -> pos_cnt)
                pm = mids.tile([P, CHUNK], F32, tag="pm")
                nc.vector.scalar_tensor_tensor(
                    pm[:], gtt[:], 0.0, mt[:], OP.bypass, OP.mult,
                    accum_out=pcnt_c[:, ch:ch + 1])
                # GPSIMD: nm = mask - pm
                nm = mids.tile([P, CHUNK], F32, tag="nm")
                nc.gpsimd.tensor_tensor(nm[:], mt[:], pm[:], OP.subtract)
                # DVE: R' = lq * nm  (resident)
                nc.vector.scalar_tensor_tensor(
                    Rp[:, sl], lq[:], 0.0, nm[:], OP.bypass, OP.mult)
                # DVE: pos-loss partial: (lp)*pm, accum -> pos_sum' (= -pos_sum)
                junkb = mids.tile([P, CHUNK], F32, tag="junkb")
                nc.vector.scalar_tensor_tensor(
                    junkb[:], lp[:], 0.0, pm[:], OP.bypass, OP.mult,
                    accum_out=psumc[:, ch:ch + 1])

            # ---- reduce per-chunk accums, pack AG1 payload ----
            pay = small.tile([P, PAY], F32)
            # sample: every 64th column of R'
            samp_view = Rp[:].rearrange("p (n s) -> p n s", s=SAMPLE_STRIDE)[:, :, 0]
            nc.vector.tensor_copy(pay[:, 0:N_SAMP], samp_view)
            nc.vector.tensor_reduce(pay[:, N_SAMP:N_SAMP + 1], pcnt_c[:],
                                    axis=mybir.AxisListType.X, op=OP.add)
            nc.vector.tensor_reduce(pay[:, N_SAMP + 1:N_SAMP + 2], psumc[:],
                                    axis=mybir.AxisListType.X, op=OP.add)
            nc.vector.tensor_reduce(pay[:, N_SAMP + 2:N_SAMP + 3], mcnt_c[:],
                                    axis=mybir.AxisListType.X, op=OP.add)
            nc.vector.memset(pay[:, N_SAMP + 3:N_SAMP + 4], 0.0)

            # ---- local pre-search on own sample: runs in the dead window
            # while the warm-up collective's ncfw setup (~70us) completes ----
            mid = small.tile([P, 1], F32)
            midt = small.tile([P, 1], F32)
            cp = small.tile([P, 1], F32)
            ge = small.tile([P, 1], F32)
            locg = small.tile([P, 8], F32)  # 0:neg_l 1:k_l 2:t_l 3:c0_l
            junkL = junk6[:, 0:N_SAMP]
            Gl = pay[:, 0:N_SAMP]
            nc.vector.tensor_tensor(locg[:, 0:1], pay[:, N_SAMP + 2:N_SAMP + 3],
                                    pay[:, N_SAMP:N_SAMP + 1], OP.subtract)
            nc.vector.tensor_scalar(locg[:, 4:5], pay[:, N_SAMP:N_SAMP + 1],
                                    NEG_RATIO, None, OP.mult)
            nc.vector.tensor_tensor(locg[:, 1:2], locg[:, 0:1], locg[:, 4:5],
                                    OP.min)
            nc.vector.tensor_scalar(junkL, Gl, -1e-3, 0.0, OP.is_lt, OP.add,
                                    accum_out=locg[:, 3:4])
            nc.vector.tensor_scalar(locg[:, 5:6], locg[:, 0:1], 1.0, None, OP.max)
            locrec = small.tile([P, 1], F32)
            nc.vector.reciprocal(locrec[:], locg[:, 5:6])
            nc.vector.tensor_tensor(locg[:, 2:3], locg[:, 1:2], locrec[:], OP.mult)
            nc.vector.tensor_tensor(locg[:, 2:3], locg[:, 2:3], locg[:, 3:4],
                                    OP.mult)
            nc.vector.memset(mid[:], LO / 2)
            step = -LO / 4
            for it in range(N_ITER):
                nc.vector.tensor_scalar(junkL, Gl, mid[:], 0.0, OP.is_lt, OP.add,
                                        accum_out=cp[:])
                nc.vector.tensor_scalar(ge[:], cp[:], locg[:, 2:3], None, OP.is_ge)
                nc.vector.scalar_tensor_tensor(midt[:], ge[:], -2.0 * step,
                                               mid[:], OP.mult, OP.add)
                nc.vector.tensor_scalar(mid[:], midt[:], step, None, OP.add)
                step *= 0.5
            # tau0 = mean over partitions of the local estimates
            pt0 = psum.tile([P, 1], F32)
            nc.tensor.matmul(pt0[:], ones[:], mid[:], start=True, stop=True)
            tau0 = small.tile([P, 1], F32)
            nc.vector.tensor_scalar(tau0[:], pt0[:], 1.0 / P, None, OP.mult)

            ag1_in = dram.tile([P, PAY], F32)
            ag1_out = dram.tile([N_CORES, P, PAY], F32, addr_space="Shared")
            nc.sync.dma_start(ag1_in[:], pay[:])
            nc.gpsimd.collective_compute(
                "AllGather", OP.bypass, replica_groups=rg,
                ins=[ag1_in.opt()], outs=[ag1_out.opt()])
            gat = small.tile([P, N_CORES * PAY], F32)
            nc.sync.dma_start(
                gat[:].rearrange("p (c j) -> p c j", c=N_CORES),
                ag1_out[:].rearrange("c p j -> p c j"))

            # views into the gathered payload
            gv = gat[:].rearrange("p (c j) -> p c j", c=N_CORES)
            G = gv[:, :, 0:N_SAMP]                 # [P, 8, 100] samples
            junkG = junk6[:, 0:N_CORES * PAY].rearrange(
                "p (c j) -> p c j", c=N_CORES)[:, :, 0:N_SAMP]

            # ---- global exact counts: pos_cnt, pos_sum', mask_sum ----
            st3 = small.tile([P, 3], F32)
            for i in range(3):
                nc.vector.tensor_reduce(st3[:, i:i + 1], gv[:, :, N_SAMP + i],
                                        axis=mybir.AxisListType.X, op=OP.add)
            ps3 = psum.tile([P, 3], F32)
            nc.tensor.matmul(ps3[:], ones[:], st3[:], start=True, stop=True)
            glob = small.tile([P, 8], F32)  # 0:pos_cnt 1:pos_sum' 2:mask_sum 3:neg_cnt 4:k 5:t_p 6:c0 7:scratch
            nc.vector.tensor_copy(glob[:, 0:3], ps3[:])
            # neg_cnt = mask_sum - pos_cnt
            nc.vector.tensor_tensor(glob[:, 3:4], glob[:, 2:3], glob[:, 0:1],
                                    OP.subtract)
            # k = min(neg_cnt, 3*pos_cnt)
            nc.vector.tensor_scalar(glob[:, 7:8], glob[:, 0:1], NEG_RATIO, None,
                                    OP.mult)
            nc.vector.tensor_tensor(glob[:, 4:5], glob[:, 3:4], glob[:, 7:8],
                                    OP.min)
            # c0_p = per-partition count of valid (nonzero) sampled negatives
            nc.vector.tensor_scalar(junkG, G, -1e-3, 0.0, OP.is_lt, OP.add,
                                    accum_out=glob[:, 6:7])
            # t_p = k * c0_p / neg_cnt
            safen = small.tile([P, 1], F32)
            nc.vector.tensor_scalar(safen[:], glob[:, 3:4], 1.0, None, OP.max)
            recn = small.tile([P, 1], F32)
            nc.vector.reciprocal(recn[:], safen[:])
            tp = small.tile([P, 1], F32)
            nc.vector.tensor_tensor(tp[:], glob[:, 4:5], recn[:], OP.mult)
            nc.vector.tensor_tensor(tp[:], tp[:], glob[:, 6:7], OP.mult)

            # ---- global refinement from tau0 on the gathered sample ----
            nc.vector.tensor_copy(mid[:], tau0[:])
            step = 0.04
            for it in range(N_REFINE):
                nc.vector.tensor_scalar(junkG, G, mid[:], 0.0, OP.is_lt, OP.add,
                                        accum_out=cp[:])
                nc.vector.tensor_scalar(ge[:], cp[:], tp[:], None, OP.is_ge)
                # mid += step * (1 - 2*ge)
                nc.vector.scalar_tensor_tensor(midt[:], ge[:], -2.0 * step,
                                               mid[:], OP.mult, OP.add)
                nc.vector.tensor_scalar(mid[:], midt[:], step, None, OP.add)
                step *= 0.5
            # tau' = mean over partitions
            pt1 = psum.tile([P, 1], F32)
            nc.tensor.matmul(pt1[:], ones[:], mid[:], start=True, stop=True)
            tau = small.tile([P, 1], F32)
            nc.vector.tensor_scalar(tau[:], pt1[:], 1.0 / P, None, OP.mult)
            ntau = small.tile([P, 1], F32)
            nc.vector.tensor_scalar(ntau[:], tau[:], -1.0, None, OP.mult)

            # ---- exact pass: S' = sum(R' [R'<tau']), sgn = sum(sign(R'-tau')) ----
            sp_c = small.tile([P, N_CH], F32)
            sg_c = small.tile([P, N_CH], F32)
            for ch in range(N_CH):
                sl = slice(ch * CHUNK, (ch + 1) * CHUNK)
                nc.vector.scalar_tensor_tensor(
                    junk6[:, sl], Rp[:, sl], tau[:], Rp[:, sl], OP.is_lt,
                    OP.mult, accum_out=sp_c[:, ch:ch + 1])
                # ACT overwrites R' chunk after the DVE pass read it
                nc.scalar.activation(Rp[:, sl], Rp[:, sl], AF.Sign,
                                     bias=ntau[:], scale=1.0,
                                     accum_out=sg_c[:, ch:ch + 1])
            fin2 = small.tile([P, 2], F32)
            nc.vector.tensor_reduce(fin2[:, 0:1], sp_c[:],
                                    axis=mybir.AxisListType.X, op=OP.add)
            nc.vector.tensor_reduce(fin2[:, 1:2], sg_c[:],
                                    axis=mybir.AxisListType.X, op=OP.add)

            ag2_in = dram.tile([P, 2], F32)
            ag2_out = dram.tile([N_CORES, P, 2], F32, addr_space="Shared")
            nc.sync.dma_start(ag2_in[:], fin2[:])
            nc.gpsimd.collective_compute(
                "AllGather", OP.bypass, replica_groups=rg,
                ins=[ag2_in.opt()], outs=[ag2_out.opt()])
            gat2 = small.tile([P, N_CORES * 2], F32)
            nc.sync.dma_start(
                gat2[:].rearrange("p (c j) -> p c j", c=N_CORES),
                ag2_out[:].rearrange("c p j -> p c j"))
            gv2 = gat2[:].rearrange("p (c j) -> p c j", c=N_CORES)
            fin2g = small.tile([P, 2], F32)
            nc.vector.tensor_reduce(fin2g[:, 0:1], gv2[:, :, 0],
                                    axis=mybir.AxisListType.X, op=OP.add)
            nc.vector.tensor_reduce(fin2g[:, 1:2], gv2[:, :, 1],
                                    axis=mybir.AxisListType.X, op=OP.add)
            pf = psum.tile([P, 2], F32)
            nc.tensor.matmul(pf[:], ones[:], fin2g[:], start=True, stop=True)

            # ---- final scalar assembly ----
            fin = small.tile([P, 8], F32)
            # C' = (N_total - sgn_g) / 2
            nc.vector.tensor_scalar(fin[:, 0:1], pf[:, 1:2], -0.5, N_TOTAL / 2,
                                    OP.mult, OP.add)
            # kmC = k - C'
            nc.vector.tensor_tensor(fin[:, 1:2], glob[:, 4:5], fin[:, 0:1],
                                    OP.subtract)
            # botk = S' + kmC * tau'
            nc.vector.tensor_tensor(fin[:, 2:3], fin[:, 1:2], tau[:], OP.mult)
            nc.vector.tensor_tensor(fin[:, 2:3], fin[:, 2:3], pf[:, 0:1], OP.add)
            # num = -(pos_sum' + botk)
            nc.vector.tensor_tensor(fin[:, 3:4], glob[:, 1:2], fin[:, 2:3], OP.add)
            nc.vector.tensor_scalar(fin[:, 3:4], fin[:, 3:4], -1.0, None, OP.mult)
            # den = pos_cnt + k + eps
            nc.vector.tensor_tensor(fin[:, 4:5], glob[:, 0:1], glob[:, 4:5], OP.add)
            nc.vector.tensor_scalar(fin[:, 4:5], fin[:, 4:5], EPS, None, OP.add)
            nc.vector.reciprocal(fin[:, 5:6], fin[:, 4:5])
            nc.vector.tensor_tensor(fin[:, 6:7], fin[:, 3:4], fin[:, 5:6], OP.mult)
            # debug row: loss, pos_cnt, neg_cnt, k, tau, S', C', num
            dbg = small.tile([1, 8], F32)
            nc.vector.tensor_copy(dbg[:, 0:1], fin[0:1, 6:7])
            nc.vector.tensor_copy(dbg[:, 1:2], glob[0:1, 0:1])
            nc.vector.tensor_copy(dbg[:, 2:3], glob[0:1, 3:4])
            nc.vector.tensor_copy(dbg[:, 3:4], glob[0:1, 4:5])
            nc.vector.tensor_copy(dbg[:, 4:5], tau[0:1, :])
            nc.vector.tensor_copy(dbg[:, 5:6], pf[0:1, 0:1])
            nc.vector.tensor_copy(dbg[:, 6:7], fin[0:1, 0:1])
            nc.vector.tensor_copy(dbg[:, 7:8], fin[0:1, 3:4])
            nc.sync.dma_start(out[:], dbg[:])
    nc.compile()
    return nc


def _get_nc():
    if "nc" not in _NC_CACHE:
        _NC_CACHE["nc"] = build()
    return _NC_CACHE["nc"]


def kernel(pred, gt, mask):
    pred = np.asarray(pred, dtype=np.float32)
    gt = np.asarray(gt, dtype=np.float32)
    mask = np.asarray(mask, dtype=np.float32)
    per = N // N_CORES
    in_maps = []
    for c in range(N_CORES):
        sl = slice(c * per, (c + 1) * per)
        in_maps.append({
            "pred": np.ascontiguousarray(pred[sl, 0].reshape(P, FREE)),
            "gt": np.ascontiguousarray(gt[sl, 0].reshape(P, FREE)),
            "mask": np.ascontiguousarray(mask[sl].reshape(P, FREE)),
        })
    nc = _get_nc()
    if TRACE:
        _ensure_trace_hook()
    res = run_bass_kernel_spmd(nc, in_maps, core_ids=list(range(N_CORES)),
                               trace=TRACE)
    kernel.last_result = res
    return np.float32(res.results[0]["out"][0, 0])


# revision 10
# speedup vs baseline: 98.5522x; 1.0573x over previous
"""BalanceCrossEntropyLoss on 8 trn2 NeuronCores.

Full (unsharded) inputs in, full output (scalar) out. Data-parallel over N:
each core takes 2 of the 16 images. The global top-k negative-loss sum is
computed threshold-style: a per-partition bisection on an all-gathered sample
estimates the k-th-largest threshold tau, then one exact masked sum/count pass
plus the correction  sum_topk = S(tau) + (k - C(tau)) * tau  (error is
quadratic in the tau estimation error; ~1e-5 relative here).
"""
import sys, types

sys.path.insert(0, "/opt/trn_rl_repo")
import numpy as np

import concourse.bass as bass
import concourse.bacc as bacc
import concourse.mybir as mybir
import concourse.tile as tile
from concourse.bass_utils import run_bass_kernel_spmd

F32 = mybir.dt.float32
OP = mybir.AluOpType
AF = mybir.ActivationFunctionType

N_CORES = 8
N, H, W = 16, 640, 640
P = 128                      # SBUF partitions
FREE = (N // N_CORES) * H * W // P   # 6400 columns per core
CHUNK = 1600                 # streaming chunk (4 chunks)
N_CH = FREE // CHUNK
SAMPLE_STRIDE = 64
N_SAMP = FREE // SAMPLE_STRIDE       # 100 sample columns per core
PAY = N_SAMP + 4             # AG1 payload cols: samples + pos_cnt, pos_sum', mask_sum, pad
N_TOTAL = float(N * H * W)   # 6553600 elements globally
NEG_RATIO = 3.0
EPS = 1e-6
# loss values -ln(1-p) lie in (0.01, 4.606] for p in [0.01, 0.99]; we search on
# negated values R' in [-4.75, 0]
LO = -4.75
N_ITER = 11
N_REFINE = 4

TRACE = False
_NC_CACHE = {}


def _ensure_trace_hook():
    import antenv
    if "antenv.axon_hooks" not in sys.modules:
        _hooks = types.ModuleType("antenv.axon_hooks")
        _hooks._hook = None
        def _set(h): _hooks._hook = h
        def _get(): return _hooks._hook
        _hooks.set_axon_ntff_profile_hook = _set
        _hooks.get_axon_ntff_profile_hook = _get
        sys.modules["antenv.axon_hooks"] = _hooks
        antenv.axon_hooks = _hooks
        from trn_agent_boot.trn_boot import _ntff_profile_via_ctypes
        _set(_ntff_profile_via_ctypes("/opt/axon/libaxon_pjrt.so"))


def build():
    nc = bacc.Bacc("TRN2", target_bir_lowering=False, debug=False,
                   num_devices=N_CORES)
    pred = nc.dram_tensor("pred", [P, FREE], F32, kind="ExternalInput").ap()
    gt = nc.dram_tensor("gt", [P, FREE], F32, kind="ExternalInput").ap()
    mask = nc.dram_tensor("mask", [P, FREE], F32, kind="ExternalInput").ap()
    out = nc.dram_tensor("out", [1, 8], F32, kind="ExternalOutput").ap()
    rg = [list(range(N_CORES))]

    with tile.TileContext(nc) as tc:
        with tc.tile_pool(name="io", bufs=2) as io, \
             tc.tile_pool(name="mids", bufs=2) as mids, \
             tc.tile_pool(name="res", bufs=1) as res, \
             tc.tile_pool(name="small", bufs=1) as small, \
             tc.tile_pool(name="psum", bufs=2, space="PSUM") as psum, \
             tc.tile_pool(name="dram", bufs=1, space="DRAM") as dram:

            # ---- warm-up collective: fires immediately (no data deps; the
            # content is irrelevant) and absorbs the ~75us first-collective
            # setup cost while streaming runs ----
            warm_in = dram.tile([P, 1], F32)
            warm_out = dram.tile([N_CORES, P, 1], F32, addr_space="Shared")
            nc.gpsimd.collective_compute(
                "AllGather", OP.bypass, replica_groups=rg,
                ins=[warm_in.opt()], outs=[warm_out.opt()])

            # ---- persistent tiles ----
            Rp = res.tile([P, FREE], F32)        # resident R' = neg * ln(1-p) <= 0
            junk6 = res.tile([P, FREE], F32)     # big scratch
            ones = small.tile([P, P], F32)
            nc.vector.memset(ones[:], 1.0)
            pcnt_c = small.tile([P, N_CH], F32)  # per-chunk accums
            psumc = small.tile([P, N_CH], F32)
            mcnt_c = small.tile([P, N_CH], F32)

            # ---- streaming phase ----
            for ch in range(N_CH):
                sl = slice(ch * CHUNK, (ch + 1) * CHUNK)
                pt = io.tile([P, CHUNK], F32, tag="pred")
                gtt = io.tile([P, CHUNK], F32, tag="gt")
                mt = io.tile([P, CHUNK], F32, tag="mask")
                nc.sync.dma_start(pt[:], pred[:, sl])
                nc.sync.dma_start(gtt[:], gt[:, sl])
                nc.sync.dma_start(mt[:], mask[:, sl])
                lp = mids.tile([P, CHUNK], F32, tag="lp")
                lq = mids.tile([P, CHUNK], F32, tag="lq")
                # ACT: ln(p), ln(1-p), and sum(mask) via Copy-accum
                nc.scalar.activation(lp[:], pt[:], AF.Ln, bias=0.0, scale=1.0)
                nc.scalar.activation(lq[:], pt[:], AF.Ln, bias=1.0, scale=-1.0)
                junka = mids.tile([P, CHUNK], F32, tag="junka")
                nc.scalar.activation(junka[:], mt[:], AF.Copy, bias=0.0,
                                     scale=1.0, accum_out=mcnt_c[:, ch:ch + 1])
                # DVE: pm = gt*mask (accum -> pos_cnt)
                pm = mids.tile([P, CHUNK], F32, tag="pm")
                nc.vector.scalar_tensor_tensor(
                    pm[:], gtt[:], 0.0, mt[:], OP.bypass, OP.mult,
                    accum_out=pcnt_c[:, ch:ch + 1])
                # GPSIMD: nm = mask - pm
                nm = mids.tile([P, CHUNK], F32, tag="nm")
                nc.gpsimd.tensor_tensor(nm[:], mt[:], pm[:], OP.subtract)
                # DVE: R' = lq * nm  (resident)
                nc.vector.scalar_tensor_tensor(
                    Rp[:, sl], lq[:], 0.0, nm[:], OP.bypass, OP.mult)
                # DVE: pos-loss partial: (lp)*pm, accum -> pos_sum' (= -pos_sum)
                junkb = mids.tile([P, CHUNK], F32, tag="junkb")
                nc.vector.scalar_tensor_tensor(
                    junkb[:], lp[:], 0.0, pm[:], OP.bypass, OP.mult,
                    accum_out=psumc[:, ch:ch + 1])

            # ---- reduce per-chunk accums, pack AG1 payload ----
            pay = small.tile([P, PAY], F32)
            # sample: every 64th column of R'
            samp_view = Rp[:].rearrange("p (n s) -> p n s", s=SAMPLE_STRIDE)[:, :, 0]
            nc.vector.tensor_copy(pay[:, 0:N_SAMP], samp_view)
            nc.vector.tensor_reduce(pay[:, N_SAMP:N_SAMP + 1], pcnt_c[:],
                                    axis=mybir.AxisListType.X, op=OP.add)
            nc.vector.tensor_reduce(pay[:, N_SAMP + 1:N_SAMP + 2], psumc[:],
                                    axis=mybir.AxisListType.X, op=OP.add)
            nc.vector.tensor_reduce(pay[:, N_SAMP + 2:N_SAMP + 3], mcnt_c[:],
                                    axis=mybir.AxisListType.X, op=OP.add)
            nc.vector.memset(pay[:, N_SAMP + 3:N_SAMP + 4], 0.0)

            # ---- local pre-search on own sample: runs in the dead window
            # while the warm-up collective's ncfw setup (~70us) completes ----
            mid = small.tile([P, 1], F32)
            midt = small.tile([P, 1], F32)
            cp = small.tile([P, 1], F32)
            ge = small.tile([P, 1], F32)
            locg = small.tile([P, 8], F32)  # 0:neg_l 1:k_l 2:t_l 3:c0_l
            junkL = junk6[:, 0:N_SAMP]
            Gl = pay[:, 0:N_SAMP]
            nc.vector.tensor_tensor(locg[:, 0:1], pay[:, N_SAMP + 2:N_SAMP + 3],
                                    pay[:, N_SAMP:N_SAMP + 1], OP.subtract)
            nc.vector.tensor_scalar(locg[:, 4:5], pay[:, N_SAMP:N_SAMP + 1],
                                    NEG_RATIO, None, OP.mult)
            nc.vector.tensor_tensor(locg[:, 1:2], locg[:, 0:1], locg[:, 4:5],
                                    OP.min)
            nc.vector.tensor_scalar(junkL, Gl, -1e-3, 0.0, OP.is_lt, OP.add,
                                    accum_out=locg[:, 3:4])
            nc.vector.tensor_scalar(locg[:, 5:6], locg[:, 0:1], 1.0, None, OP.max)
            locrec = small.tile([P, 1], F32)
            nc.vector.reciprocal(locrec[:], locg[:, 5:6])
            nc.vector.tensor_tensor(locg[:, 2:3], locg[:, 1:2], locrec[:], OP.mult)
            nc.vector.tensor_tensor(locg[:, 2:3], locg[:, 2:3], locg[:, 3:4],
                                    OP.mult)
            nc.vector.memset(mid[:], LO / 2)
            step = -LO / 4
            for it in range(N_ITER):
                nc.vector.tensor_scalar(junkL, Gl, mid[:], 0.0, OP.is_lt, OP.add,
                                        accum_out=cp[:])
                nc.vector.tensor_scalar(ge[:], cp[:], locg[:, 2:3], None, OP.is_ge)
                nc.vector.scalar_tensor_tensor(midt[:], ge[:], -2.0 * step,
                                               mid[:], OP.mult, OP.add)
                nc.vector.tensor_scalar(mid[:], midt[:], step, None, OP.add)
                step *= 0.5
            # tau0 = mean over partitions of the local estimates
            pt0 = psum.tile([P, 1], F32)
            nc.tensor.matmul(pt0[:], ones[:], mid[:], start=True, stop=True)
            tau0 = small.tile([P, 1], F32)
            nc.vector.tensor_scalar(tau0[:], pt0[:], 1.0 / P, None, OP.mult)

            # the exact pass runs at this core's own tau0; the correction
            # formula tolerates per-core thresholds (error ~ sum_c m_c*dtau_c^2)
            ntau = small.tile([P, 1], F32)
            nc.vector.tensor_scalar(ntau[:], tau0[:], -1.0, None, OP.mult)

            # ---- exact pass: S' = sum(R' [R'<tau']), sgn = sum(sign(R'-tau')) ----
            sp_c = small.tile([P, N_CH], F32)
            sg_c = small.tile([P, N_CH], F32)
            for ch in range(N_CH):
                sl = slice(ch * CHUNK, (ch + 1) * CHUNK)
                nc.vector.scalar_tensor_tensor(
                    junk6[:, sl], Rp[:, sl], tau0[:], Rp[:, sl], OP.is_lt,
                    OP.mult, accum_out=sp_c[:, ch:ch + 1])
                # ACT overwrites R' chunk after the DVE pass read it
                nc.scalar.activation(Rp[:, sl], Rp[:, sl], AF.Sign,
                                     bias=ntau[:], scale=1.0,
                                     accum_out=sg_c[:, ch:ch + 1])
            fin2 = small.tile([P, 8], F32)
            nc.vector.tensor_reduce(fin2[:, 0:1], sp_c[:],
                                    axis=mybir.AxisListType.X, op=OP.add)
            nc.vector.tensor_reduce(fin2[:, 1:2], sg_c[:],
                                    axis=mybir.AxisListType.X, op=OP.add)
            nc.vector.tensor_copy(fin2[:, 2:5], pay[:, N_SAMP:N_SAMP + 3])
            nc.vector.tensor_copy(fin2[:, 5:6], tau0[:])
            nc.vector.memset(fin2[:, 6:8], 0.0)

            ag2_in = dram.tile([P, 8], F32)
            ag2_out = dram.tile([N_CORES, P, 8], F32, addr_space="Shared")
            nc.sync.dma_start(ag2_in[:], fin2[:])
            nc.gpsimd.collective_compute(
                "AllGather", OP.bypass, replica_groups=rg,
                ins=[ag2_in.opt()], outs=[ag2_out.opt()])
            gat2 = small.tile([P, N_CORES * 8], F32)
            nc.sync.dma_start(
                gat2[:].rearrange("p (c j) -> p c j", c=N_CORES),
                ag2_out[:].rearrange("c p j -> p c j"))
            gv2 = gat2[:].rearrange("p (c j) -> p c j", c=N_CORES)
            fin2g = small.tile([P, 6], F32)
            for i in range(6):
                nc.vector.tensor_reduce(fin2g[:, i:i + 1], gv2[:, :, i],
                                        axis=mybir.AxisListType.X, op=OP.add)
            # pf cols: 0 S'_g, 1 sgn_g, 2 pos_cnt_g, 3 pos_sum'_g, 4 mask_g, 5 1024*tau_bar
            pfp = psum.tile([P, 6], F32)
            nc.tensor.matmul(pfp[:], ones[:], fin2g[:], start=True, stop=True)
            pf = small.tile([P, 6], F32)
            nc.vector.tensor_copy(pf[:], pfp[:])

            # ---- final scalar assembly ----
            fin = small.tile([P, 8], F32)
            glob = small.tile([P, 8], F32)  # 0 pos_cnt 1 neg_cnt 2 k 3 tau_bar
            nc.vector.tensor_copy(glob[:, 0:1], pf[:, 2:3])
            nc.vector.tensor_tensor(glob[:, 1:2], pf[:, 4:5], pf[:, 2:3],
                                    OP.subtract)
            nc.vector.tensor_scalar(glob[:, 4:5], pf[:, 2:3], NEG_RATIO, None,
                                    OP.mult)
            nc.vector.tensor_tensor(glob[:, 2:3], glob[:, 1:2], glob[:, 4:5],
                                    OP.min)
            nc.vector.tensor_scalar(glob[:, 3:4], pf[:, 5:6], 1.0 / (P * N_CORES),
                                    None, OP.mult)
            # C' = (N_total - sgn_g) / 2
            nc.vector.tensor_scalar(fin[:, 0:1], pf[:, 1:2], -0.5, N_TOTAL / 2,
                                    OP.mult, OP.add)
            # kmC = k - C'
            nc.vector.tensor_tensor(fin[:, 1:2], glob[:, 2:3], fin[:, 0:1],
                                    OP.subtract)
            # botk = S'_g + kmC * tau_bar
            nc.vector.tensor_tensor(fin[:, 2:3], fin[:, 1:2], glob[:, 3:4], OP.mult)
            nc.vector.tensor_tensor(fin[:, 2:3], fin[:, 2:3], pf[:, 0:1], OP.add)
            # num = -(pos_sum' + botk)
            nc.vector.tensor_tensor(fin[:, 3:4], pf[:, 3:4], fin[:, 2:3], OP.add)
            nc.vector.tensor_scalar(fin[:, 3:4], fin[:, 3:4], -1.0, None, OP.mult)
            # den = pos_cnt + k + eps
            nc.vector.tensor_tensor(fin[:, 4:5], glob[:, 0:1], glob[:, 2:3], OP.add)
            nc.vector.tensor_scalar(fin[:, 4:5], fin[:, 4:5], EPS, None, OP.add)
            nc.vector.reciprocal(fin[:, 5:6], fin[:, 4:5])
            nc.vector.tensor_tensor(fin[:, 6:7], fin[:, 3:4], fin[:, 5:6], OP.mult)
            # debug row: loss, pos_cnt, neg_cnt, k, tau, S', C', num
            dbg = small.tile([1, 8], F32)
            nc.vector.tensor_copy(dbg[:, 0:1], fin[0:1, 6:7])
            nc.vector.tensor_copy(dbg[:, 1:2], glob[0:1, 0:1])
            nc.vector.tensor_copy(dbg[:, 2:3], glob[0:1, 1:2])
            nc.vector.tensor_copy(dbg[:, 3:4], glob[0:1, 2:3])
            nc.vector.tensor_copy(dbg[:, 4:5], glob[0:1, 3:4])
            nc.vector.tensor_copy(dbg[:, 5:6], pf[0:1, 0:1])
            nc.vector.tensor_copy(dbg[:, 6:7], fin[0:1, 0:1])
            nc.vector.tensor_copy(dbg[:, 7:8], fin[0:1, 3:4])
            nc.sync.dma_start(out[:], dbg[:])
    nc.compile()
    return nc


def _get_nc():
    if "nc" not in _NC_CACHE:
        _NC_CACHE["nc"] = build()
    return _NC_CACHE["nc"]


def kernel(pred, gt, mask):
    pred = np.asarray(pred, dtype=np.float32)
    gt = np.asarray(gt, dtype=np.float32)
    mask = np.asarray(mask, dtype=np.float32)
    per = N // N_CORES
    in_maps = []
    for c in range(N_CORES):
        sl = slice(c * per, (c + 1) * per)
        in_maps.append({
            "pred": np.ascontiguousarray(pred[sl, 0].reshape(P, FREE)),
            "gt": np.ascontiguousarray(gt[sl, 0].reshape(P, FREE)),
            "mask": np.ascontiguousarray(mask[sl].reshape(P, FREE)),
        })
    nc = _get_nc()
    if TRACE:
        _ensure_trace_hook()
    res = run_bass_kernel_spmd(nc, in_maps, core_ids=list(range(N_CORES)),
                               trace=TRACE)
    kernel.last_result = res
    return np.float32(res.results[0]["out"][0, 0])


# revision 11
# speedup vs baseline: 113.4788x; 1.1515x over previous
"""BalanceCrossEntropyLoss on 8 trn2 NeuronCores.

Full (unsharded) inputs in, full output (scalar) out. Data-parallel over N:
each core takes 2 of the 16 images. The global top-k negative-loss sum is
computed threshold-style: a per-partition bisection on an all-gathered sample
estimates the k-th-largest threshold tau, then one exact masked sum/count pass
plus the correction  sum_topk = S(tau) + (k - C(tau)) * tau  (error is
quadratic in the tau estimation error; ~1e-5 relative here).
"""
import sys, types

sys.path.insert(0, "/opt/trn_rl_repo")
import numpy as np

import concourse.bass as bass
import concourse.bacc as bacc
import concourse.mybir as mybir
import concourse.tile as tile
from concourse.bass_utils import run_bass_kernel_spmd

F32 = mybir.dt.float32
OP = mybir.AluOpType
AF = mybir.ActivationFunctionType

N_CORES = 8
N, H, W = 16, 640, 640
P = 128                      # SBUF partitions
FREE = (N // N_CORES) * H * W // P   # 6400 columns per core
CHUNK = 1600                 # streaming chunk (4 chunks)
N_CH = FREE // CHUNK
SAMPLE_STRIDE = 64
N_SAMP = FREE // SAMPLE_STRIDE       # 100 sample columns per core
PAY = N_SAMP + 4             # AG1 payload cols: samples + pos_cnt, pos_sum', mask_sum, pad
N_TOTAL = float(N * H * W)   # 6553600 elements globally
NEG_RATIO = 3.0
EPS = 1e-6
# loss values -ln(1-p) lie in (0.01, 4.606] for p in [0.01, 0.99]; we search on
# negated values R' in [-4.75, 0]
LO = -4.75
N_ITER = 11
N_REFINE = 4

TRACE = False
_NC_CACHE = {}


def _ensure_trace_hook():
    import antenv
    if "antenv.axon_hooks" not in sys.modules:
        _hooks = types.ModuleType("antenv.axon_hooks")
        _hooks._hook = None
        def _set(h): _hooks._hook = h
        def _get(): return _hooks._hook
        _hooks.set_axon_ntff_profile_hook = _set
        _hooks.get_axon_ntff_profile_hook = _get
        sys.modules["antenv.axon_hooks"] = _hooks
        antenv.axon_hooks = _hooks
        from trn_agent_boot.trn_boot import _ntff_profile_via_ctypes
        _set(_ntff_profile_via_ctypes("/opt/axon/libaxon_pjrt.so"))


def build():
    nc = bacc.Bacc("TRN2", target_bir_lowering=False, debug=False,
                   num_devices=N_CORES)
    pred = nc.dram_tensor("pred", [P, FREE], F32, kind="ExternalInput").ap()
    gt = nc.dram_tensor("gt", [P, FREE], F32, kind="ExternalInput").ap()
    mask = nc.dram_tensor("mask", [P, FREE], F32, kind="ExternalInput").ap()
    out = nc.dram_tensor("out", [1, 8], F32, kind="ExternalOutput").ap()
    rg = [list(range(N_CORES))]

    with tile.TileContext(nc) as tc:
        with tc.tile_pool(name="io", bufs=2) as io, \
             tc.tile_pool(name="mids", bufs=2) as mids, \
             tc.tile_pool(name="res", bufs=1) as res, \
             tc.tile_pool(name="small", bufs=1) as small, \
             tc.tile_pool(name="psum", bufs=2, space="PSUM") as psum, \
             tc.tile_pool(name="dram", bufs=1, space="DRAM") as dram:

            # ---- warm-up collective: fires immediately (no data deps; the
            # content is irrelevant) and absorbs the ~75us first-collective
            # setup cost while streaming runs ----
            warm_in = dram.tile([P, 1], F32)
            warm_out = dram.tile([1, P, 1], F32, addr_space="Shared")
            nc.gpsimd.collective_compute(
                "AllGather", OP.bypass,
                replica_groups=[[c] for c in range(N_CORES)],
                ins=[warm_in.opt()], outs=[warm_out.opt()])

            # ---- persistent tiles ----
            Rp = res.tile([P, FREE], F32)        # resident R' = neg * ln(1-p) <= 0
            junk6 = res.tile([P, FREE], F32)     # big scratch
            ones = small.tile([P, P], F32)
            nc.vector.memset(ones[:], 1.0)
            pcnt_c = small.tile([P, N_CH], F32)  # per-chunk accums
            psumc = small.tile([P, N_CH], F32)
            mcnt_c = small.tile([P, N_CH], F32)

            # ---- streaming phase ----
            for ch in range(N_CH):
                sl = slice(ch * CHUNK, (ch + 1) * CHUNK)
                pt = io.tile([P, CHUNK], F32, tag="pred")
                gtt = io.tile([P, CHUNK], F32, tag="gt")
                mt = io.tile([P, CHUNK], F32, tag="mask")
                nc.sync.dma_start(pt[:], pred[:, sl])
                nc.sync.dma_start(gtt[:], gt[:, sl])
                nc.sync.dma_start(mt[:], mask[:, sl])
                lp = mids.tile([P, CHUNK], F32, tag="lp")
                lq = mids.tile([P, CHUNK], F32, tag="lq")
                # ACT: ln(p), ln(1-p), and sum(mask) via Copy-accum
                nc.scalar.activation(lp[:], pt[:], AF.Ln, bias=0.0, scale=1.0)
                nc.scalar.activation(lq[:], pt[:], AF.Ln, bias=1.0, scale=-1.0)
                junka = mids.tile([P, CHUNK], F32, tag="junka")
                nc.scalar.activation(junka[:], mt[:], AF.Copy, bias=0.0,
                                     scale=1.0, accum_out=mcnt_c[:, ch:ch + 1])
                # DVE: pm = gt*mask (accum -> pos_cnt)
                pm = mids.tile([P, CHUNK], F32, tag="pm")
                nc.vector.scalar_tensor_tensor(
                    pm[:], gtt[:], 0.0, mt[:], OP.bypass, OP.mult,
                    accum_out=pcnt_c[:, ch:ch + 1])
                # GPSIMD: nm = mask - pm
                nm = mids.tile([P, CHUNK], F32, tag="nm")
                nc.gpsimd.tensor_tensor(nm[:], mt[:], pm[:], OP.subtract)
                # DVE: R' = lq * nm  (resident)
                nc.vector.scalar_tensor_tensor(
                    Rp[:, sl], lq[:], 0.0, nm[:], OP.bypass, OP.mult)
                # DVE: pos-loss partial: (lp)*pm, accum -> pos_sum' (= -pos_sum)
                junkb = mids.tile([P, CHUNK], F32, tag="junkb")
                nc.vector.scalar_tensor_tensor(
                    junkb[:], lp[:], 0.0, pm[:], OP.bypass, OP.mult,
                    accum_out=psumc[:, ch:ch + 1])

            # ---- reduce per-chunk accums, pack AG1 payload ----
            pay = small.tile([P, PAY], F32)
            # sample: every 64th column of R'
            samp_view = Rp[:].rearrange("p (n s) -> p n s", s=SAMPLE_STRIDE)[:, :, 0]
            nc.vector.tensor_copy(pay[:, 0:N_SAMP], samp_view)
            nc.vector.tensor_reduce(pay[:, N_SAMP:N_SAMP + 1], pcnt_c[:],
                                    axis=mybir.AxisListType.X, op=OP.add)
            nc.vector.tensor_reduce(pay[:, N_SAMP + 1:N_SAMP + 2], psumc[:],
                                    axis=mybir.AxisListType.X, op=OP.add)
            nc.vector.tensor_reduce(pay[:, N_SAMP + 2:N_SAMP + 3], mcnt_c[:],
                                    axis=mybir.AxisListType.X, op=OP.add)
            nc.vector.memset(pay[:, N_SAMP + 3:N_SAMP + 4], 0.0)

            # ---- local pre-search on own sample: runs in the dead window
            # while the warm-up collective's ncfw setup (~70us) completes ----
            mid = small.tile([P, 1], F32)
            midt = small.tile([P, 1], F32)
            cp = small.tile([P, 1], F32)
            ge = small.tile([P, 1], F32)
            locg = small.tile([P, 8], F32)  # 0:neg_l 1:k_l 2:t_l 3:c0_l
            junkL = junk6[:, 0:N_SAMP]
            Gl = pay[:, 0:N_SAMP]
            nc.vector.tensor_tensor(locg[:, 0:1], pay[:, N_SAMP + 2:N_SAMP + 3],
                                    pay[:, N_SAMP:N_SAMP + 1], OP.subtract)
            nc.vector.tensor_scalar(locg[:, 4:5], pay[:, N_SAMP:N_SAMP + 1],
                                    NEG_RATIO, None, OP.mult)
            nc.vector.tensor_tensor(locg[:, 1:2], locg[:, 0:1], locg[:, 4:5],
                                    OP.min)
            nc.vector.tensor_scalar(junkL, Gl, -1e-3, 0.0, OP.is_lt, OP.add,
                                    accum_out=locg[:, 3:4])
            nc.vector.tensor_scalar(locg[:, 5:6], locg[:, 0:1], 1.0, None, OP.max)
            locrec = small.tile([P, 1], F32)
            nc.vector.reciprocal(locrec[:], locg[:, 5:6])
            nc.vector.tensor_tensor(locg[:, 2:3], locg[:, 1:2], locrec[:], OP.mult)
            nc.vector.tensor_tensor(locg[:, 2:3], locg[:, 2:3], locg[:, 3:4],
                                    OP.mult)
            nc.vector.memset(mid[:], LO / 2)
            step = -LO / 4
            for it in range(N_ITER):
                nc.vector.tensor_scalar(junkL, Gl, mid[:], 0.0, OP.is_lt, OP.add,
                                        accum_out=cp[:])
                nc.vector.tensor_scalar(ge[:], cp[:], locg[:, 2:3], None, OP.is_ge)
                nc.vector.scalar_tensor_tensor(midt[:], ge[:], -2.0 * step,
                                               mid[:], OP.mult, OP.add)
                nc.vector.tensor_scalar(mid[:], midt[:], step, None, OP.add)
                step *= 0.5
            # tau0 = mean over partitions of the local estimates
            pt0 = psum.tile([P, 1], F32)
            nc.tensor.matmul(pt0[:], ones[:], mid[:], start=True, stop=True)
            tau0 = small.tile([P, 1], F32)
            nc.vector.tensor_scalar(tau0[:], pt0[:], 1.0 / P, None, OP.mult)

            # the exact pass runs at this core's own tau0; the correction
            # formula tolerates per-core thresholds (error ~ sum_c m_c*dtau_c^2)
            ntau = small.tile([P, 1], F32)
            nc.vector.tensor_scalar(ntau[:], tau0[:], -1.0, None, OP.mult)

            # ---- exact pass: S' = sum(R' [R'<tau']), sgn = sum(sign(R'-tau')) ----
            sp_c = small.tile([P, N_CH], F32)
            sg_c = small.tile([P, N_CH], F32)
            for ch in range(N_CH):
                sl = slice(ch * CHUNK, (ch + 1) * CHUNK)
                nc.vector.scalar_tensor_tensor(
                    junk6[:, sl], Rp[:, sl], tau0[:], Rp[:, sl], OP.is_lt,
                    OP.mult, accum_out=sp_c[:, ch:ch + 1])
                # ACT overwrites R' chunk after the DVE pass read it
                nc.scalar.activation(Rp[:, sl], Rp[:, sl], AF.Sign,
                                     bias=ntau[:], scale=1.0,
                                     accum_out=sg_c[:, ch:ch + 1])
            fin2 = small.tile([P, 8], F32)
            nc.vector.tensor_reduce(fin2[:, 0:1], sp_c[:],
                                    axis=mybir.AxisListType.X, op=OP.add)
            nc.vector.tensor_reduce(fin2[:, 1:2], sg_c[:],
                                    axis=mybir.AxisListType.X, op=OP.add)
            nc.vector.tensor_copy(fin2[:, 2:5], pay[:, N_SAMP:N_SAMP + 3])
            nc.vector.tensor_copy(fin2[:, 5:6], tau0[:])
            nc.vector.memset(fin2[:, 6:8], 0.0)

            # partition-reduce before the collective: payload is [1,8] (32B)
            pfp = psum.tile([P, 8], F32)
            nc.tensor.matmul(pfp[:], ones[:], fin2[:], start=True, stop=True)
            row8 = small.tile([1, 8], F32)
            nc.vector.tensor_copy(row8[:], pfp[0:1, :])
            ag2_in = dram.tile([1, 8], F32)
            ag2_out = dram.tile([N_CORES, 1, 8], F32, addr_space="Shared")
            nc.sync.dma_start(ag2_in[:], row8[:])
            nc.gpsimd.collective_compute(
                "AllGather", OP.bypass, replica_groups=rg,
                ins=[ag2_in.opt()], outs=[ag2_out.opt()])
            g64 = small.tile([1, N_CORES * 8], F32)
            nc.sync.dma_start(
                g64[:].rearrange("p (c j) -> p c j", c=N_CORES),
                ag2_out[:].rearrange("c p j -> p c j"))
            # pf[0, j] = sum over cores of stat j
            pf = small.tile([1, 8], F32)
            nc.vector.tensor_reduce(
                pf[:], g64[:].rearrange("p (c j) -> p j c", c=N_CORES),
                axis=mybir.AxisListType.X, op=OP.add)

            # ---- final scalar assembly (single partition) ----
            # pf cols: 0 S'_g 1 sgn_g 2 pos_cnt 3 pos_sum' 4 mask_sum 5 1024*tau_bar
            fin = small.tile([1, 8], F32)
            glob = small.tile([1, 8], F32)  # 0 pos_cnt 1 neg_cnt 2 k 3 tau_bar
            nc.vector.tensor_copy(glob[:, 0:1], pf[:, 2:3])
            nc.vector.tensor_tensor(glob[:, 1:2], pf[:, 4:5], pf[:, 2:3],
                                    OP.subtract)
            nc.vector.tensor_scalar(glob[:, 4:5], pf[:, 2:3], NEG_RATIO, None,
                                    OP.mult)
            nc.vector.tensor_tensor(glob[:, 2:3], glob[:, 1:2], glob[:, 4:5],
                                    OP.min)
            nc.vector.tensor_scalar(glob[:, 3:4], pf[:, 5:6], 1.0 / (P * N_CORES),
                                    None, OP.mult)
            # C' = (N_total - sgn_g) / 2 ; kmC = k - C'
            nc.vector.tensor_scalar(fin[:, 0:1], pf[:, 1:2], -0.5, N_TOTAL / 2,
                                    OP.mult, OP.add)
            nc.vector.tensor_tensor(fin[:, 1:2], glob[:, 2:3], fin[:, 0:1],
                                    OP.subtract)
            # botk = S'_g + kmC * tau_bar
            nc.vector.tensor_tensor(fin[:, 2:3], fin[:, 1:2], glob[:, 3:4], OP.mult)
            nc.vector.tensor_tensor(fin[:, 2:3], fin[:, 2:3], pf[:, 0:1], OP.add)
            # num = -(pos_sum' + botk) ; den = pos_cnt + k + eps
            nc.vector.tensor_tensor(fin[:, 3:4], pf[:, 3:4], fin[:, 2:3], OP.add)
            nc.vector.tensor_scalar(fin[:, 3:4], fin[:, 3:4], -1.0, None, OP.mult)
            nc.vector.tensor_tensor(fin[:, 4:5], glob[:, 0:1], glob[:, 2:3], OP.add)
            nc.vector.tensor_scalar(fin[:, 4:5], fin[:, 4:5], EPS, None, OP.add)
            nc.vector.reciprocal(fin[:, 5:6], fin[:, 4:5])
            nc.vector.tensor_tensor(fin[:, 6:7], fin[:, 3:4], fin[:, 5:6], OP.mult)
            # debug row: loss, pos_cnt, neg_cnt, k, tau, S', C', num
            dbg = small.tile([1, 8], F32)
            nc.vector.tensor_copy(dbg[:, 0:1], fin[:, 6:7])
            nc.vector.tensor_copy(dbg[:, 1:2], glob[:, 0:1])
            nc.vector.tensor_copy(dbg[:, 2:3], glob[:, 1:2])
            nc.vector.tensor_copy(dbg[:, 3:4], glob[:, 2:3])
            nc.vector.tensor_copy(dbg[:, 4:5], glob[:, 3:4])
            nc.vector.tensor_copy(dbg[:, 5:6], pf[:, 0:1])
            nc.vector.tensor_copy(dbg[:, 6:7], fin[:, 0:1])
            nc.vector.tensor_copy(dbg[:, 7:8], fin[:, 3:4])
            nc.sync.dma_start(out[:], dbg[:])
    nc.compile()
    return nc


def _get_nc():
    if "nc" not in _NC_CACHE:
        _NC_CACHE["nc"] = build()
    return _NC_CACHE["nc"]


def kernel(pred, gt, mask):
    pred = np.asarray(pred, dtype=np.float32)
    gt = np.asarray(gt, dtype=np.float32)
    mask = np.asarray(mask, dtype=np.float32)
    per = N // N_CORES
    in_maps = []
    for c in range(N_CORES):
        sl = slice(c * per, (c + 1) * per)
        in_maps.append({
            "pred": np.ascontiguousarray(pred[sl, 0].reshape(P, FREE)),
            "gt": np.ascontiguousarray(gt[sl, 0].reshape(P, FREE)),
            "mask": np.ascontiguousarray(mask[sl].reshape(P, FREE)),
        })
    nc = _get_nc()
    if TRACE:
        _ensure_trace_hook()
    res = run_bass_kernel_spmd(nc, in_maps, core_ids=list(range(N_CORES)),
                               trace=TRACE)
    kernel.last_result = res
    return np.float32(res.results[0]["out"][0, 0])


# revision 12
# speedup vs baseline: 123.1070x; 1.0848x over previous
"""BalanceCrossEntropyLoss on 8 trn2 NeuronCores.

Full (unsharded) inputs in, full output (scalar) out. Data-parallel over N:
each core takes 2 of the 16 images. The global top-k negative-loss sum is
computed threshold-style: a per-partition bisection on an all-gathered sample
estimates the k-th-largest threshold tau, then one exact masked sum/count pass
plus the correction  sum_topk = S(tau) + (k - C(tau)) * tau  (error is
quadratic in the tau estimation error; ~1e-5 relative here).
"""
import sys, types

sys.path.insert(0, "/opt/trn_rl_repo")
import numpy as np

import concourse.bass as bass
import concourse.bacc as bacc
import concourse.mybir as mybir
import concourse.tile as tile
from concourse.bass_utils import run_bass_kernel_spmd

F32 = mybir.dt.float32
OP = mybir.AluOpType
AF = mybir.ActivationFunctionType

N_CORES = 8
N, H, W = 16, 640, 640
P = 128                      # SBUF partitions
FREE = (N // N_CORES) * H * W // P   # 6400 columns per core
CHUNK = 1600                 # streaming chunk (4 chunks)
N_CH = FREE // CHUNK
SAMPLE_STRIDE = 64
N_SAMP = FREE // SAMPLE_STRIDE       # 100 sample columns per core
PAY = N_SAMP + 4             # AG1 payload cols: samples + pos_cnt, pos_sum', mask_sum, pad
N_TOTAL = float(N * H * W)   # 6553600 elements globally
NEG_RATIO = 3.0
EPS = 1e-6
# loss values -ln(1-p) lie in (0.01, 4.606] for p in [0.01, 0.99]; we search on
# negated values R' in [-4.75, 0]
LO = -4.75
N_ITER = 9
N_REFINE = 4

TRACE = False
_NC_CACHE = {}


def _ensure_trace_hook():
    import antenv
    if "antenv.axon_hooks" not in sys.modules:
        _hooks = types.ModuleType("antenv.axon_hooks")
        _hooks._hook = None
        def _set(h): _hooks._hook = h
        def _get(): return _hooks._hook
        _hooks.set_axon_ntff_profile_hook = _set
        _hooks.get_axon_ntff_profile_hook = _get
        sys.modules["antenv.axon_hooks"] = _hooks
        antenv.axon_hooks = _hooks
        from trn_agent_boot.trn_boot import _ntff_profile_via_ctypes
        _set(_ntff_profile_via_ctypes("/opt/axon/libaxon_pjrt.so"))


def build():
    nc = bacc.Bacc("TRN2", target_bir_lowering=False, debug=False,
                   num_devices=N_CORES)
    pred = nc.dram_tensor("pred", [P, FREE], F32, kind="ExternalInput").ap()
    gt = nc.dram_tensor("gt", [P, FREE], F32, kind="ExternalInput").ap()
    mask = nc.dram_tensor("mask", [P, FREE], F32, kind="ExternalInput").ap()
    out = nc.dram_tensor("out", [1, 8], F32, kind="ExternalOutput").ap()
    rg = [list(range(N_CORES))]

    with tile.TileContext(nc) as tc:
        with tc.tile_pool(name="io", bufs=2) as io, \
             tc.tile_pool(name="mids", bufs=2) as mids, \
             tc.tile_pool(name="res", bufs=1) as res, \
             tc.tile_pool(name="small", bufs=1) as small, \
             tc.tile_pool(name="psum", bufs=2, space="PSUM") as psum, \
             tc.tile_pool(name="dram", bufs=1, space="DRAM") as dram:

            # ---- warm-up collective: fires immediately (no data deps; the
            # content is irrelevant) and absorbs the ~75us first-collective
            # setup cost while streaming runs ----
            warm_in = dram.tile([P, 1], F32)
            warm_out = dram.tile([1, P, 1], F32, addr_space="Shared")
            nc.gpsimd.collective_compute(
                "AllGather", OP.bypass,
                replica_groups=[[c] for c in range(N_CORES)],
                ins=[warm_in.opt()], outs=[warm_out.opt()])

            # ---- persistent tiles ----
            Rp = res.tile([P, FREE], F32)        # resident R' = neg * ln(1-p) <= 0
            junk6 = res.tile([P, FREE], F32)     # big scratch
            ones = small.tile([P, P], F32)
            nc.vector.memset(ones[:], 1.0)
            pcnt_c = small.tile([P, N_CH], F32)  # per-chunk accums
            psumc = small.tile([P, N_CH], F32)
            mcnt_c = small.tile([P, N_CH], F32)

            # ---- streaming phase ----
            for ch in range(N_CH):
                sl = slice(ch * CHUNK, (ch + 1) * CHUNK)
                pt = io.tile([P, CHUNK], F32, tag="pred")
                gtt = io.tile([P, CHUNK], F32, tag="gt")
                mt = io.tile([P, CHUNK], F32, tag="mask")
                nc.sync.dma_start(pt[:], pred[:, sl])
                nc.sync.dma_start(gtt[:], gt[:, sl])
                nc.sync.dma_start(mt[:], mask[:, sl])
                lp = mids.tile([P, CHUNK], F32, tag="lp")
                lq = mids.tile([P, CHUNK], F32, tag="lq")
                # ACT: ln(p), ln(1-p), and sum(mask) via Copy-accum
                nc.scalar.activation(lp[:], pt[:], AF.Ln, bias=0.0, scale=1.0)
                nc.scalar.activation(lq[:], pt[:], AF.Ln, bias=1.0, scale=-1.0)
                junka = mids.tile([P, CHUNK], F32, tag="junka")
                nc.scalar.activation(junka[:], mt[:], AF.Copy, bias=0.0,
                                     scale=1.0, accum_out=mcnt_c[:, ch:ch + 1])
                # DVE: pm = gt*mask (accum -> pos_cnt)
                pm = mids.tile([P, CHUNK], F32, tag="pm")
                nc.vector.scalar_tensor_tensor(
                    pm[:], gtt[:], 0.0, mt[:], OP.bypass, OP.mult,
                    accum_out=pcnt_c[:, ch:ch + 1])
                # nm = mask - pm: alternate GpSimd/DVE per chunk (GpSimd alone
                # is the stream bottleneck at ~5us per 2-input pass)
                nm = mids.tile([P, CHUNK], F32, tag="nm")
                if ch % 2 == 0:
                    nc.gpsimd.tensor_tensor(nm[:], mt[:], pm[:], OP.subtract)
                else:
                    nc.vector.scalar_tensor_tensor(
                        nm[:], pm[:], -1.0, mt[:], OP.mult, OP.add)
                # DVE: R' = lq * nm  (resident)
                nc.vector.scalar_tensor_tensor(
                    Rp[:, sl], lq[:], 0.0, nm[:], OP.bypass, OP.mult)
                # DVE: pos-loss partial: (lp)*pm, accum -> pos_sum' (= -pos_sum)
                junkb = mids.tile([P, CHUNK], F32, tag="junkb")
                nc.vector.scalar_tensor_tensor(
                    junkb[:], lp[:], 0.0, pm[:], OP.bypass, OP.mult,
                    accum_out=psumc[:, ch:ch + 1])

            # ---- reduce per-chunk accums, pack AG1 payload ----
            pay = small.tile([P, PAY], F32)
            # sample: every 64th column of R'
            samp_view = Rp[:].rearrange("p (n s) -> p n s", s=SAMPLE_STRIDE)[:, :, 0]
            nc.vector.tensor_copy(pay[:, 0:N_SAMP], samp_view)
            nc.vector.tensor_reduce(pay[:, N_SAMP:N_SAMP + 1], pcnt_c[:],
                                    axis=mybir.AxisListType.X, op=OP.add)
            nc.vector.tensor_reduce(pay[:, N_SAMP + 1:N_SAMP + 2], psumc[:],
                                    axis=mybir.AxisListType.X, op=OP.add)
            nc.vector.tensor_reduce(pay[:, N_SAMP + 2:N_SAMP + 3], mcnt_c[:],
                                    axis=mybir.AxisListType.X, op=OP.add)
            nc.vector.memset(pay[:, N_SAMP + 3:N_SAMP + 4], 0.0)

            # ---- local pre-search on own sample: runs in the dead window
            # while the warm-up collective's ncfw setup (~70us) completes ----
            mid = small.tile([P, 1], F32)
            midt = small.tile([P, 1], F32)
            cp = small.tile([P, 1], F32)
            ge = small.tile([P, 1], F32)
            locg = small.tile([P, 8], F32)  # 0:neg_l 1:k_l 2:t_l 3:c0_l
            junkL = junk6[:, 0:N_SAMP]
            Gl = pay[:, 0:N_SAMP]
            nc.vector.tensor_tensor(locg[:, 0:1], pay[:, N_SAMP + 2:N_SAMP + 3],
                                    pay[:, N_SAMP:N_SAMP + 1], OP.subtract)
            nc.vector.tensor_scalar(locg[:, 4:5], pay[:, N_SAMP:N_SAMP + 1],
                                    NEG_RATIO, None, OP.mult)
            nc.vector.tensor_tensor(locg[:, 1:2], locg[:, 0:1], locg[:, 4:5],
                                    OP.min)
            nc.vector.tensor_scalar(junkL, Gl, -1e-3, 0.0, OP.is_lt, OP.add,
                                    accum_out=locg[:, 3:4])
            nc.vector.tensor_scalar(locg[:, 5:6], locg[:, 0:1], 1.0, None, OP.max)
            locrec = small.tile([P, 1], F32)
            nc.vector.reciprocal(locrec[:], locg[:, 5:6])
            nc.vector.tensor_tensor(locg[:, 2:3], locg[:, 1:2], locrec[:], OP.mult)
            nc.vector.tensor_tensor(locg[:, 2:3], locg[:, 2:3], locg[:, 3:4],
                                    OP.mult)
            nc.vector.memset(mid[:], LO / 2)
            step = -LO / 4
            for it in range(N_ITER):
                nc.vector.tensor_scalar(junkL, Gl, mid[:], 0.0, OP.is_lt, OP.add,
                                        accum_out=cp[:])
                nc.vector.tensor_scalar(ge[:], cp[:], locg[:, 2:3], None, OP.is_ge)
                nc.vector.scalar_tensor_tensor(midt[:], ge[:], -2.0 * step,
                                               mid[:], OP.mult, OP.add)
                nc.vector.tensor_scalar(mid[:], midt[:], step, None, OP.add)
                step *= 0.5
            # tau0 = mean over partitions of the local estimates
            pt0 = psum.tile([P, 1], F32)
            nc.tensor.matmul(pt0[:], ones[:], mid[:], start=True, stop=True)
            tau0 = small.tile([P, 1], F32)
            nc.vector.tensor_scalar(tau0[:], pt0[:], 1.0 / P, None, OP.mult)

            # the exact pass runs at this core's own tau0; the correction
            # formula tolerates per-core thresholds (error ~ sum_c m_c*dtau_c^2)
            ntau = small.tile([P, 1], F32)
            nc.vector.tensor_scalar(ntau[:], tau0[:], -1.0, None, OP.mult)

            # ---- exact pass: S' = sum(R' [R'<tau']), sgn = sum(sign(R'-tau')) ----
            sp_c = small.tile([P, N_CH], F32)
            sg_c = small.tile([P, N_CH], F32)
            for ch in range(N_CH):
                sl = slice(ch * CHUNK, (ch + 1) * CHUNK)
                nc.vector.scalar_tensor_tensor(
                    junk6[:, sl], Rp[:, sl], tau0[:], Rp[:, sl], OP.is_lt,
                    OP.mult, accum_out=sp_c[:, ch:ch + 1])
                # ACT overwrites R' chunk after the DVE pass read it
                nc.scalar.activation(Rp[:, sl], Rp[:, sl], AF.Sign,
                                     bias=ntau[:], scale=1.0,
                                     accum_out=sg_c[:, ch:ch + 1])
            fin2 = small.tile([P, 8], F32)
            nc.vector.tensor_reduce(fin2[:, 0:1], sp_c[:],
                                    axis=mybir.AxisListType.X, op=OP.add)
            nc.vector.tensor_reduce(fin2[:, 1:2], sg_c[:],
                                    axis=mybir.AxisListType.X, op=OP.add)
            nc.vector.tensor_copy(fin2[:, 2:5], pay[:, N_SAMP:N_SAMP + 3])
            nc.vector.tensor_copy(fin2[:, 5:6], tau0[:])
            nc.vector.memset(fin2[:, 6:8], 0.0)

            # partition-reduce before the collective: payload is [1,8] (32B)
            pfp = psum.tile([P, 8], F32)
            nc.tensor.matmul(pfp[:], ones[:], fin2[:], start=True, stop=True)
            row8 = small.tile([1, 8], F32)
            nc.vector.tensor_copy(row8[:], pfp[0:1, :])
            ag2_in = dram.tile([1, 8], F32)
            ag2_out = dram.tile([N_CORES, 1, 8], F32, addr_space="Shared")
            nc.sync.dma_start(ag2_in[:], row8[:])
            nc.gpsimd.collective_compute(
                "AllGather", OP.bypass, replica_groups=rg,
                ins=[ag2_in.opt()], outs=[ag2_out.opt()])
            g64 = small.tile([1, N_CORES * 8], F32)
            nc.sync.dma_start(
                g64[:].rearrange("p (c j) -> p c j", c=N_CORES),
                ag2_out[:].rearrange("c p j -> p c j"))
            # pf[0, j] = sum over cores of stat j
            pf = small.tile([1, 8], F32)
            nc.vector.tensor_reduce(
                pf[:], g64[:].rearrange("p (c j) -> p j c", c=N_CORES),
                axis=mybir.AxisListType.X, op=OP.add)

            # ---- final scalar assembly (single partition) ----
            # pf cols: 0 S'_g 1 sgn_g 2 pos_cnt 3 pos_sum' 4 mask_sum 5 1024*tau_bar
            fin = small.tile([1, 8], F32)
            glob = small.tile([1, 8], F32)  # 0 pos_cnt 1 neg_cnt 2 k 3 tau_bar
            nc.vector.tensor_copy(glob[:, 0:1], pf[:, 2:3])
            nc.vector.tensor_tensor(glob[:, 1:2], pf[:, 4:5], pf[:, 2:3],
                                    OP.subtract)
            nc.vector.tensor_scalar(glob[:, 4:5], pf[:, 2:3], NEG_RATIO, None,
                                    OP.mult)
            nc.vector.tensor_tensor(glob[:, 2:3], glob[:, 1:2], glob[:, 4:5],
                                    OP.min)
            nc.vector.tensor_scalar(glob[:, 3:4], pf[:, 5:6], 1.0 / (P * N_CORES),
                                    None, OP.mult)
            # C' = (N_total - sgn_g) / 2 ; kmC = k - C'
            nc.vector.tensor_scalar(fin[:, 0:1], pf[:, 1:2], -0.5, N_TOTAL / 2,
                                    OP.mult, OP.add)
            nc.vector.tensor_tensor(fin[:, 1:2], glob[:, 2:3], fin[:, 0:1],
                                    OP.subtract)
            # botk = S'_g + kmC * tau_bar
            nc.vector.tensor_tensor(fin[:, 2:3], fin[:, 1:2], glob[:, 3:4], OP.mult)
            nc.vector.tensor_tensor(fin[:, 2:3], fin[:, 2:3], pf[:, 0:1], OP.add)
            # num = -(pos_sum' + botk) ; den = pos_cnt + k + eps
            nc.vector.tensor_tensor(fin[:, 3:4], pf[:, 3:4], fin[:, 2:3], OP.add)
            nc.vector.tensor_scalar(fin[:, 3:4], fin[:, 3:4], -1.0, None, OP.mult)
            nc.vector.tensor_tensor(fin[:, 4:5], glob[:, 0:1], glob[:, 2:3], OP.add)
            nc.vector.tensor_scalar(fin[:, 4:5], fin[:, 4:5], EPS, None, OP.add)
            nc.vector.reciprocal(fin[:, 5:6], fin[:, 4:5])
            nc.vector.tensor_tensor(fin[:, 6:7], fin[:, 3:4], fin[:, 5:6], OP.mult)
            # debug row: loss, pos_cnt, neg_cnt, k, tau, S', C', num
            dbg = small.tile([1, 8], F32)
            nc.vector.tensor_copy(dbg[:, 0:1], fin[:, 6:7])
            nc.vector.tensor_copy(dbg[:, 1:2], glob[:, 0:1])
            nc.vector.tensor_copy(dbg[:, 2:3], glob[:, 1:2])
            nc.vector.tensor_copy(dbg[:, 3:4], glob[:, 2:3])
            nc.vector.tensor_copy(dbg[:, 4:5], glob[:, 3:4])
            nc.vector.tensor_copy(dbg[:, 5:6], pf[:, 0:1])
            nc.vector.tensor_copy(dbg[:, 6:7], fin[:, 0:1])
            nc.vector.tensor_copy(dbg[:, 7:8], fin[:, 3:4])
            nc.sync.dma_start(out[:], dbg[:])
    nc.compile()
    return nc


def _get_nc():
    if "nc" not in _NC_CACHE:
        _NC_CACHE["nc"] = build()
    return _NC_CACHE["nc"]


def kernel(pred, gt, mask):
    pred = np.asarray(pred, dtype=np.float32)
    gt = np.asarray(gt, dtype=np.float32)
    mask = np.asarray(mask, dtype=np.float32)
    per = N // N_CORES
    in_maps = []
    for c in range(N_CORES):
        sl = slice(c * per, (c + 1) * per)
        in_maps.append({
            "pred": np.ascontiguousarray(pred[sl, 0].reshape(P, FREE)),
            "gt": np.ascontiguousarray(gt[sl, 0].reshape(P, FREE)),
            "mask": np.ascontiguousarray(mask[sl].reshape(P, FREE)),
        })
    nc = _get_nc()
    if TRACE:
        _ensure_trace_hook()
    res = run_bass_kernel_spmd(nc, in_maps, core_ids=list(range(N_CORES)),
                               trace=TRACE)
    kernel.last_result = res
    return np.float32(res.results[0]["out"][0, 0])


# revision 13
# speedup vs baseline: 132.4120x; 1.0756x over previous
"""BalanceCrossEntropyLoss on 8 trn2 NeuronCores.

Full (unsharded) inputs in, full output (scalar) out. Data-parallel over N:
each core takes 2 of the 16 images. The global top-k negative-loss sum is
computed threshold-style: a per-partition bisection on an all-gathered sample
estimates the k-th-largest threshold tau, then one exact masked sum/count pass
plus the correction  sum_topk = S(tau) + (k - C(tau)) * tau  (error is
quadratic in the tau estimation error; ~1e-5 relative here).
"""
import sys, types

sys.path.insert(0, "/opt/trn_rl_repo")
import numpy as np

import concourse.bass as bass
import concourse.bacc as bacc
import concourse.mybir as mybir
import concourse.tile as tile
from concourse.bass_utils import run_bass_kernel_spmd

F32 = mybir.dt.float32
OP = mybir.AluOpType
AF = mybir.ActivationFunctionType

N_CORES = 8
N, H, W = 16, 640, 640
P = 128                      # SBUF partitions
FREE = (N // N_CORES) * H * W // P   # 6400 columns per core
CHUNK = 1600                 # streaming chunk (4 chunks)
N_CH = FREE // CHUNK
SAMPLE_STRIDE = 64
N_SAMP = FREE // SAMPLE_STRIDE       # 100 sample columns per core
PAY = N_SAMP + 4             # AG1 payload cols: samples + pos_cnt, pos_sum', mask_sum, pad
N_TOTAL = float(N * H * W)   # 6553600 elements globally
NEG_RATIO = 3.0
EPS = 1e-6
# loss values -ln(1-p) lie in (0.01, 4.606] for p in [0.01, 0.99]; we search on
# negated values R' in [-4.75, 0]
LO = -4.75
N_ITER = 9
N_REFINE = 4

TRACE = False
_NC_CACHE = {}


def _ensure_trace_hook():
    import antenv
    if "antenv.axon_hooks" not in sys.modules:
        _hooks = types.ModuleType("antenv.axon_hooks")
        _hooks._hook = None
        def _set(h): _hooks._hook = h
        def _get(): return _hooks._hook
        _hooks.set_axon_ntff_profile_hook = _set
        _hooks.get_axon_ntff_profile_hook = _get
        sys.modules["antenv.axon_hooks"] = _hooks
        antenv.axon_hooks = _hooks
        from trn_agent_boot.trn_boot import _ntff_profile_via_ctypes
        _set(_ntff_profile_via_ctypes("/opt/axon/libaxon_pjrt.so"))


def build():
    nc = bacc.Bacc("TRN2", target_bir_lowering=False, debug=False,
                   num_devices=N_CORES)
    pred = nc.dram_tensor("pred", [P, FREE], F32, kind="ExternalInput").ap()
    gt = nc.dram_tensor("gt", [P, FREE], F32, kind="ExternalInput").ap()
    mask = nc.dram_tensor("mask", [P, FREE], F32, kind="ExternalInput").ap()
    out = nc.dram_tensor("out", [1, 8], F32, kind="ExternalOutput").ap()
    rg = [list(range(N_CORES))]

    with tile.TileContext(nc) as tc:
        with tc.tile_pool(name="io", bufs=2) as io, \
             tc.tile_pool(name="mids", bufs=2) as mids, \
             tc.tile_pool(name="res", bufs=1) as res, \
             tc.tile_pool(name="small", bufs=1) as small, \
             tc.tile_pool(name="psum", bufs=2, space="PSUM") as psum, \
             tc.tile_pool(name="dram", bufs=1, space="DRAM") as dram:

            # ---- warm-up collective: fires immediately (no data deps; the
            # content is irrelevant) and absorbs the ~75us first-collective
            # setup cost while streaming runs ----
            warm_in = dram.tile([P, 1], F32)
            warm_out = dram.tile([1, P, 1], F32, addr_space="Shared")
            nc.gpsimd.collective_compute(
                "AllGather", OP.bypass,
                replica_groups=[[c] for c in range(N_CORES)],
                ins=[warm_in.opt()], outs=[warm_out.opt()])

            # ---- persistent tiles ----
            Rp = res.tile([P, FREE], F32)        # resident R' = neg * ln(1-p) <= 0
            junk6 = res.tile([P, FREE], F32)     # big scratch
            ones = small.tile([P, P], F32)
            nc.vector.memset(ones[:], 1.0)
            pcnt_c = small.tile([P, N_CH], F32)  # per-chunk accums
            psumc = small.tile([P, N_CH], F32)
            mcnt_c = small.tile([P, N_CH], F32)

            # ---- streaming phase ----
            for ch in range(N_CH):
                sl = slice(ch * CHUNK, (ch + 1) * CHUNK)
                pt = io.tile([P, CHUNK], F32, tag="pred")
                gtt = io.tile([P, CHUNK], F32, tag="gt")
                mt = io.tile([P, CHUNK], F32, tag="mask")
                nc.sync.dma_start(pt[:], pred[:, sl])
                nc.sync.dma_start(gtt[:], gt[:, sl])
                nc.sync.dma_start(mt[:], mask[:, sl])
                lp = mids.tile([P, CHUNK], F32, tag="lp")
                lq = mids.tile([P, CHUNK], F32, tag="lq")
                # ACT: ln(p), ln(1-p), and sum(mask) via Copy-accum
                nc.scalar.activation(lp[:], pt[:], AF.Ln, bias=0.0, scale=1.0)
                nc.scalar.activation(lq[:], pt[:], AF.Ln, bias=1.0, scale=-1.0)
                junka = mids.tile([P, CHUNK], F32, tag="junka")
                nc.scalar.activation(junka[:], mt[:], AF.Copy, bias=0.0,
                                     scale=1.0, accum_out=mcnt_c[:, ch:ch + 1])
                # DVE: pm = gt*mask (accum -> pos_cnt)
                pm = mids.tile([P, CHUNK], F32, tag="pm")
                nc.vector.scalar_tensor_tensor(
                    pm[:], gtt[:], 0.0, mt[:], OP.bypass, OP.mult,
                    accum_out=pcnt_c[:, ch:ch + 1])
                # nm = mask - pm: alternate GpSimd/DVE per chunk (GpSimd alone
                # is the stream bottleneck at ~5us per 2-input pass)
                nm = mids.tile([P, CHUNK], F32, tag="nm")
                if ch % 2 == 0:
                    nc.gpsimd.tensor_tensor(nm[:], mt[:], pm[:], OP.subtract)
                else:
                    nc.vector.scalar_tensor_tensor(
                        nm[:], pm[:], -1.0, mt[:], OP.mult, OP.add)
                # DVE: R' = lq * nm  (resident)
                nc.vector.scalar_tensor_tensor(
                    Rp[:, sl], lq[:], 0.0, nm[:], OP.bypass, OP.mult)
                # DVE: pos-loss partial: (lp)*pm, accum -> pos_sum' (= -pos_sum)
                junkb = mids.tile([P, CHUNK], F32, tag="junkb")
                nc.vector.scalar_tensor_tensor(
                    junkb[:], lp[:], 0.0, pm[:], OP.bypass, OP.mult,
                    accum_out=psumc[:, ch:ch + 1])

            # ---- reduce per-chunk accums, pack AG1 payload ----
            pay = small.tile([P, PAY], F32)
            # sample: every 64th column of R'
            samp_view = Rp[:].rearrange("p (n s) -> p n s", s=SAMPLE_STRIDE)[:, :, 0]
            nc.vector.tensor_copy(pay[:, 0:N_SAMP], samp_view)
            nc.vector.tensor_reduce(pay[:, N_SAMP:N_SAMP + 1], pcnt_c[:],
                                    axis=mybir.AxisListType.X, op=OP.add)
            nc.vector.tensor_reduce(pay[:, N_SAMP + 1:N_SAMP + 2], psumc[:],
                                    axis=mybir.AxisListType.X, op=OP.add)
            nc.vector.tensor_reduce(pay[:, N_SAMP + 2:N_SAMP + 3], mcnt_c[:],
                                    axis=mybir.AxisListType.X, op=OP.add)
            nc.vector.memset(pay[:, N_SAMP + 3:N_SAMP + 4], 0.0)

            # ---- local pre-search on own sample: runs in the dead window
            # while the warm-up collective's ncfw setup (~70us) completes ----
            mid = small.tile([P, 1], F32)
            midt = small.tile([P, 1], F32)
            cp = small.tile([P, 1], F32)
            ge = small.tile([P, 1], F32)
            locg = small.tile([P, 8], F32)  # 0:neg_l 1:k_l 2:t_l 3:c0_l
            junkL = junk6[:, 0:N_SAMP]
            Gl = pay[:, 0:N_SAMP]
            nc.vector.tensor_tensor(locg[:, 0:1], pay[:, N_SAMP + 2:N_SAMP + 3],
                                    pay[:, N_SAMP:N_SAMP + 1], OP.subtract)
            nc.vector.tensor_scalar(locg[:, 4:5], pay[:, N_SAMP:N_SAMP + 1],
                                    NEG_RATIO, None, OP.mult)
            nc.vector.tensor_tensor(locg[:, 1:2], locg[:, 0:1], locg[:, 4:5],
                                    OP.min)
            nc.vector.tensor_scalar(junkL, Gl, -1e-3, 0.0, OP.is_lt, OP.add,
                                    accum_out=locg[:, 3:4])
            nc.vector.tensor_scalar(locg[:, 5:6], locg[:, 0:1], 1.0, None, OP.max)
            locrec = small.tile([P, 1], F32)
            nc.vector.reciprocal(locrec[:], locg[:, 5:6])
            nc.vector.tensor_tensor(locg[:, 2:3], locg[:, 1:2], locrec[:], OP.mult)
            nc.vector.tensor_tensor(locg[:, 2:3], locg[:, 2:3], locg[:, 3:4],
                                    OP.mult)
            nc.vector.memset(mid[:], LO / 2)
            step = -LO / 4
            for it in range(N_ITER):
                nc.vector.tensor_scalar(junkL, Gl, mid[:], 0.0, OP.is_lt, OP.add,
                                        accum_out=cp[:])
                nc.vector.tensor_scalar(ge[:], cp[:], locg[:, 2:3], None, OP.is_ge)
                nc.vector.scalar_tensor_tensor(midt[:], ge[:], -2.0 * step,
                                               mid[:], OP.mult, OP.add)
                nc.vector.tensor_scalar(mid[:], midt[:], step, None, OP.add)
                step *= 0.5
            # tau0 = mean over partitions of the local estimates
            pt0 = psum.tile([P, 1], F32)
            nc.tensor.matmul(pt0[:], ones[:], mid[:], start=True, stop=True)
            tau0 = small.tile([P, 1], F32)
            nc.vector.tensor_scalar(tau0[:], pt0[:], 1.0 / P, None, OP.mult)

            # the exact pass runs at this core's own tau0; the correction
            # formula tolerates per-core thresholds (error ~ sum_c m_c*dtau_c^2)
            ntau = small.tile([P, 1], F32)
            nc.vector.tensor_scalar(ntau[:], tau0[:], -1.0, None, OP.mult)

            # ---- exact pass: S' = sum(R' [R'<tau']), sgn = sum(sign(R'-tau')) ----
            sp_c = small.tile([P, N_CH], F32)
            sg_c = small.tile([P, N_CH], F32)
            for ch in range(N_CH):
                sl = slice(ch * CHUNK, (ch + 1) * CHUNK)
                nc.vector.scalar_tensor_tensor(
                    junk6[:, sl], Rp[:, sl], tau0[:], Rp[:, sl], OP.is_lt,
                    OP.mult, accum_out=sp_c[:, ch:ch + 1])
                # ACT overwrites R' chunk after the DVE pass read it
                nc.scalar.activation(Rp[:, sl], Rp[:, sl], AF.Sign,
                                     bias=ntau[:], scale=1.0,
                                     accum_out=sg_c[:, ch:ch + 1])
            fin2 = small.tile([P, 8], F32)
            nc.vector.tensor_reduce(fin2[:, 0:1], sp_c[:],
                                    axis=mybir.AxisListType.X, op=OP.add)
            nc.vector.tensor_reduce(fin2[:, 1:2], sg_c[:],
                                    axis=mybir.AxisListType.X, op=OP.add)
            nc.vector.tensor_copy(fin2[:, 2:5], pay[:, N_SAMP:N_SAMP + 3])
            nc.vector.tensor_copy(fin2[:, 5:6], tau0[:])
            nc.vector.memset(fin2[:, 6:8], 0.0)

            # partition-reduce before the collective: payload is [1,8] (32B)
            pfp = psum.tile([P, 8], F32)
            nc.tensor.matmul(pfp[:], ones[:], fin2[:], start=True, stop=True)
            row8 = small.tile([1, 8], F32)
            nc.vector.tensor_copy(row8[:], pfp[0:1, :])
            ag2_in = dram.tile([1, 8], F32)
            ag2_out = dram.tile([1, 8], F32, addr_space="Shared")
            nc.sync.dma_start(ag2_in[:], row8[:])
            nc.gpsimd.collective_compute(
                "AllReduce", OP.add, replica_groups=rg,
                ins=[ag2_in.opt()], outs=[ag2_out.opt()])
            pf = small.tile([1, 8], F32)
            nc.sync.dma_start(pf[:], ag2_out[:])

            # ---- final scalar assembly (single partition) ----
            # pf cols: 0 S'_g 1 sgn_g 2 pos_cnt 3 pos_sum' 4 mask_sum 5 1024*tau_bar
            fin = small.tile([1, 8], F32)
            glob = small.tile([1, 8], F32)  # 0 pos_cnt 1 neg_cnt 2 k 3 tau_bar
            nc.vector.tensor_copy(glob[:, 0:1], pf[:, 2:3])
            nc.vector.tensor_tensor(glob[:, 1:2], pf[:, 4:5], pf[:, 2:3],
                                    OP.subtract)
            nc.vector.tensor_scalar(glob[:, 4:5], pf[:, 2:3], NEG_RATIO, None,
                                    OP.mult)
            nc.vector.tensor_tensor(glob[:, 2:3], glob[:, 1:2], glob[:, 4:5],
                                    OP.min)
            nc.vector.tensor_scalar(glob[:, 3:4], pf[:, 5:6], 1.0 / (P * N_CORES),
                                    None, OP.mult)
            # C' = (N_total - sgn_g) / 2 ; kmC = k - C'
            nc.vector.tensor_scalar(fin[:, 0:1], pf[:, 1:2], -0.5, N_TOTAL / 2,
                                    OP.mult, OP.add)
            nc.vector.tensor_tensor(fin[:, 1:2], glob[:, 2:3], fin[:, 0:1],
                                    OP.subtract)
            # botk = S'_g + kmC * tau_bar
            nc.vector.tensor_tensor(fin[:, 2:3], fin[:, 1:2], glob[:, 3:4], OP.mult)
            nc.vector.tensor_tensor(fin[:, 2:3], fin[:, 2:3], pf[:, 0:1], OP.add)
            # num = -(pos_sum' + botk) ; den = pos_cnt + k + eps
            nc.vector.tensor_tensor(fin[:, 3:4], pf[:, 3:4], fin[:, 2:3], OP.add)
            nc.vector.tensor_scalar(fin[:, 3:4], fin[:, 3:4], -1.0, None, OP.mult)
            nc.vector.tensor_tensor(fin[:, 4:5], glob[:, 0:1], glob[:, 2:3], OP.add)
            nc.vector.tensor_scalar(fin[:, 4:5], fin[:, 4:5], EPS, None, OP.add)
            nc.vector.reciprocal(fin[:, 5:6], fin[:, 4:5])
            nc.vector.tensor_tensor(fin[:, 6:7], fin[:, 3:4], fin[:, 5:6], OP.mult)
            # debug row: loss, pos_cnt, neg_cnt, k, tau, S', C', num
            dbg = small.tile([1, 8], F32)
            nc.vector.tensor_copy(dbg[:, 0:1], fin[:, 6:7])
            nc.vector.tensor_copy(dbg[:, 1:2], glob[:, 0:1])
            nc.vector.tensor_copy(dbg[:, 2:3], glob[:, 1:2])
            nc.vector.tensor_copy(dbg[:, 3:4], glob[:, 2:3])
            nc.vector.tensor_copy(dbg[:, 4:5], glob[:, 3:4])
            nc.vector.tensor_copy(dbg[:, 5:6], pf[:, 0:1])
            nc.vector.tensor_copy(dbg[:, 6:7], fin[:, 0:1])
            nc.vector.tensor_copy(dbg[:, 7:8], fin[:, 3:4])
            nc.sync.dma_start(out[:], dbg[:])
    nc.compile()
    return nc


def _get_nc():
    if "nc" not in _NC_CACHE:
        _NC_CACHE["nc"] = build()
    return _NC_CACHE["nc"]


def kernel(pred, gt, mask):
    pred = np.asarray(pred, dtype=np.float32)
    gt = np.asarray(gt, dtype=np.float32)
    mask = np.asarray(mask, dtype=np.float32)
    per = N // N_CORES
    in_maps = []
    for c in range(N_CORES):
        sl = slice(c * per, (c + 1) * per)
        in_maps.append({
            "pred": np.ascontiguousarray(pred[sl, 0].reshape(P, FREE)),
            "gt": np.ascontiguousarray(gt[sl, 0].reshape(P, FREE)),
            "mask": np.ascontiguousarray(mask[sl].reshape(P, FREE)),
        })
    nc = _get_nc()
    if TRACE:
        _ensure_trace_hook()
    res = run_bass_kernel_spmd(nc, in_maps, core_ids=list(range(N_CORES)),
                               trace=TRACE)
    kernel.last_result = res
    return np.float32(res.results[0]["out"][0, 0])
